# revision 1
# baseline (speedup 1.0000x reference)
"""3-layer GAT on 8 Trainium2 NeuronCores (Bass/Tile).

Edge-sharded by destination range:
  - Nodes split into 8 contiguous ranges (one per core); each core owns the
    softmax + aggregation for its destination nodes.
  - Per layer a packed per-node table [h | a_src] (c-major feature order) is
    computed locally and AllGathered (bf16, 768B rows); a_dst lives in a
    per-core local table (256B rows).
  - Edges (with self loops) are bucketed per core into 128-dst tiles x
    128-edge chunks; chunk structure (incl. lo/hi int16-index table halves)
    is made identical across cores so one SPMD instruction stream fits all.
  - Per 8-chunk super-batch the kernel dma_gathers source rows + dest
    attention rows, computes w = exp(leaky_relu(a_src+a_dst)) (softmax
    shift-invariance removes the segment-max pass at these value ranges),
    scales messages by w, and segment-sums with matmuls against one-hot
    membership matrices (tensor_scalar is_equal vs an iota tile), keeping
    numerator and denominator together in PSUM.  The per-tile epilogue
    divides, applies bias/relu, and feeds the next layer's matmul whose rhs
    [W | W@att_src | W@att_dst] also emits the next attention scores.
"""

import numpy as np
import ml_dtypes

N = 50000
E = 800000
IN_C = 128
HID = 32
OUT_C = 40
HEADS = 8
NEG_SLOPE = 0.2
NCORES = 8

_BF16 = ml_dtypes.bfloat16

KSUP = 8  # chunks per gather super-batch (1024 idx = dma_gather limit)


def _cmajor_perm(heads, ch):
    f_new = np.arange(heads * ch)
    return (f_new % heads) * ch + f_new // heads  # perm[new] = old


def _attn_cols(w, att):
    heads, ch = att.shape
    return np.einsum("khc,hc->kh", w.reshape(-1, heads, ch), att).astype(np.float32)


def _prep_weights(W1, as1, ad1, b1, W2, as2, ad2, b2, W3, as3, ad3, b3):
    W1 = np.asarray(W1, np.float32)
    W2 = np.asarray(W2, np.float32)
    W3 = np.asarray(W3, np.float32)
    perm = _cmajor_perm(HEADS, HID)

    rhs1 = np.concatenate(
        [W1[:, perm], _attn_cols(W1, np.asarray(as1, np.float32)),
         _attn_cols(W1, np.asarray(ad1, np.float32))], axis=1).astype(np.float32)
    W2r = W2[perm, :]
    rhs2 = np.concatenate(
        [W2r[:, perm], _attn_cols(W2r, np.asarray(as2, np.float32)),
         _attn_cols(W2r, np.asarray(ad2, np.float32))], axis=1).astype(np.float32)
    W3r = W3[perm, :]
    as3p = (W3r @ np.asarray(as3, np.float32)[0]).reshape(-1, 1)
    ad3p = (W3r @ np.asarray(ad3, np.float32)[0]).reshape(-1, 1)
    rhs3 = np.concatenate([W3r, as3p, ad3p], axis=1).astype(np.float32)

    def bcast(b):
        return np.tile(np.asarray(b, np.float32)[None, :], (128, 1))

    return (rhs1, rhs2, rhs3,
            bcast(np.asarray(b1, np.float32)[perm]),
            bcast(np.asarray(b2, np.float32)[perm]),
            bcast(np.asarray(b3, np.float32)))


def _prep_graph(edge_index):
    """Slot edges into the SPMD-uniform (tile, section, chunk) grid."""
    src = np.concatenate([edge_index[0], np.arange(N)]).astype(np.int64)
    dst = np.concatenate([edge_index[1], np.arange(N)]).astype(np.int64)

    npc = N // NCORES
    ntiles = (npc + 127) // 128
    nmax = ntiles * 128
    half = (NCORES // 2) * nmax

    core_of = dst // npc
    d_loc = dst - core_of * npc
    tile_of = d_loc // 128
    s_core = src // npc
    s_row = s_core * nmax + (src - s_core * npc)  # AG-table row of src
    is_hi = s_row >= half

    cnt = np.zeros((NCORES, ntiles, 2), np.int64)
    np.add.at(cnt, (core_of, tile_of, is_hi.astype(np.int64)), 1)
    sec_cpt = np.ceil(cnt / 128).astype(np.int64).max(axis=0)  # [ntiles, 2]
    sec_cpt[:, 0] = np.maximum(sec_cpt[:, 0], 1)

    total = int(sec_cpt.sum())
    pad = (-total) % KSUP
    sec_cpt[-1, 1] += pad
    total += pad
    nsup = total // KSUP

    tile_of_chunk = []
    tag_of_chunk = []
    for t in range(ntiles):
        tile_of_chunk += [t] * int(sec_cpt[t, 0] + sec_cpt[t, 1])
        tag_of_chunk += [0] * int(sec_cpt[t, 0]) + [1] * int(sec_cpt[t, 1])
    tile_of_chunk = np.array(tile_of_chunk)
    tag_of_chunk = np.array(tag_of_chunk)
    sec_base = np.zeros((ntiles, 2), np.int64)
    sec_base.ravel()[1:] = np.cumsum(sec_cpt.ravel())[:-1]

    src_w = np.zeros((NCORES, 128, total * 8), np.int16)
    adst_w = np.zeros((NCORES, 128, total * 8), np.int16)
    seg = np.full((NCORES, nsup, 128, KSUP), 255.0, np.float32)

    order = np.lexsort((src, is_hi, tile_of, core_of))
    src_o = s_row[order]
    dst_o = d_loc[order]
    core_o = core_of[order]
    tile_o = tile_of[order]
    hi_o = is_hi[order]

    for k in range(NCORES):
        m = core_o == k
        t = tile_o[m]
        hi = hi_o[m].astype(np.int64)
        sr = src_o[m] - hi * half
        dl = dst_o[m]
        key = t * 2 + hi
        cnts = np.bincount(key, minlength=ntiles * 2)
        st = np.zeros(ntiles * 2, np.int64)
        st[1:] = np.cumsum(cnts)[:-1]
        pos = np.arange(len(t)) - st[key]
        q = sec_base[t, hi] + pos // 128
        p = pos % 128
        col = q * 8 + p // 16
        row = p % 16
        for c in range(8):
            src_w[k, row + 16 * c, col] = sr
            adst_w[k, row + 16 * c, col] = dl
        seg[k, q // KSUP, p, q % KSUP] = (dl % 128).astype(np.float32)

    runs = []  # (sup, chunk_lo, chunk_hi, tag)
    for s in range(nsup):
        q0 = s * KSUP
        r0 = q0
        for q in range(q0 + 1, q0 + KSUP + 1):
            if q == q0 + KSUP or tag_of_chunk[q] != tag_of_chunk[r0]:
                runs.append((s, r0, q, int(tag_of_chunk[r0])))
                r0 = q

    return dict(
        src_w=src_w, adst_w=adst_w, seg=seg,
        tile_of_chunk=tile_of_chunk, runs=runs, nsup=nsup, total=total,
        ntiles=ntiles, nmax=nmax, npc=npc, half=half,
    )


def _build_bass(g, repeat=1):
    import concourse.bacc as bacc
    import concourse.mybir as mybir
    import concourse.tile as tile
    from concourse.masks import make_identity

    dt = mybir.dt
    Alu = mybir.AluOpType
    Act = mybir.ActivationFunctionType

    ntiles, nmax, nsup, total = g["ntiles"], g["nmax"], g["nsup"], g["total"]
    half = g["half"]
    tile_of_chunk = g["tile_of_chunk"]
    H2 = HEADS * HID  # 256
    PACK = H2 + 2 * HEADS  # 272 psum width: h + a_src + a_dst
    TW = 384  # AG table row width (768B)
    TW3 = 128  # layer-3 / a_dst table row width (256B)
    GW = H2 + HEADS  # 264 useful gathered cols
    GW3 = OUT_C + 1  # 41

    first_chunk = {}
    last_chunk = {}
    for q, t in enumerate(tile_of_chunk):
        first_chunk.setdefault(int(t), q)
        last_chunk[int(t)] = q
    runs_by_sup = {}
    for (s, a, b, tag) in g["runs"]:
        runs_by_sup.setdefault(s, []).append((a, b, tag))

    nc = bacc.Bacc("TRN2", target_bir_lowering=False, debug=False,
                   num_devices=NCORES, num_swdge_queues=1)

    xT = nc.dram_tensor("xT", [IN_C, nmax], dt.float32, kind="ExternalInput")
    rhs1 = nc.dram_tensor("rhs1", [IN_C, PACK], dt.float32, kind="ExternalInput")
    rhs2 = nc.dram_tensor("rhs2", [H2, PACK], dt.float32, kind="ExternalInput")
    rhs3 = nc.dram_tensor("rhs3", [H2, OUT_C + 2], dt.float32,
                          kind="ExternalInput")
    b1r = nc.dram_tensor("b1r", [128, H2], dt.float32, kind="ExternalInput")
    b2r = nc.dram_tensor("b2r", [128, H2], dt.float32, kind="ExternalInput")
    b3r = nc.dram_tensor("b3r", [128, OUT_C], dt.float32, kind="ExternalInput")
    iota = nc.dram_tensor("iota", [128, 128], dt.bfloat16, kind="ExternalInput")
    src_w = nc.dram_tensor("src_w", [128, total * 8], dt.int16,
                           kind="ExternalInput")
    adst_w = nc.dram_tensor("adst_w", [128, total * 8], dt.int16,
                            kind="ExternalInput")
    seg_in = nc.dram_tensor("seg", [nsup, 128, KSUP], dt.float32,
                            kind="ExternalInput")
    out = nc.dram_tensor("out", [nmax, OUT_C], dt.float32,
                         kind="ExternalOutput")


    with tile.TileContext(nc) as tc:
        with (
            tc.tile_pool(name="const", bufs=1) as constp,
            tc.tile_pool(name="sbuf", bufs=4) as sbuf,
            tc.tile_pool(name="gbuf", bufs=4) as gbuf,
            tc.tile_pool(name="mbuf", bufs=4) as mbuf,
            tc.tile_pool(name="epil", bufs=2) as epil,
            tc.tile_pool(name="psum_seg", bufs=3, space="PSUM") as psum_seg,
            tc.tile_pool(name="psum_h", bufs=2, space="PSUM") as psum_h,
            tc.tile_pool(name="psum_tp", bufs=2, space="PSUM") as psum_tp,
            tc.tile_pool(name="dram", bufs=1, space="DRAM") as dram,
        ):
            # ---- constants ----
            xT_s = constp.tile([IN_C, nmax], dt.float32)
            nc.sync.dma_start(out=xT_s[:], in_=xT[:])
            rhs1_s = constp.tile([IN_C, PACK], dt.float32)
            nc.sync.dma_start(out=rhs1_s[:], in_=rhs1[:])
            rhs2_s = constp.tile([128, 2 * PACK], dt.float32)
            nc.sync.dma_start(
                out=rhs2_s[:].rearrange("p (k f) -> p k f", k=2),
                in_=rhs2[:].rearrange("(k p) f -> p k f", p=128))
            rhs3_s = constp.tile([128, 2 * (OUT_C + 2)], dt.float32)
            nc.sync.dma_start(
                out=rhs3_s[:].rearrange("p (k f) -> p k f", k=2),
                in_=rhs3[:].rearrange("(k p) f -> p k f", p=128))
            b1_s = constp.tile([128, H2], dt.float32)
            nc.sync.dma_start(out=b1_s[:], in_=b1r[:])
            b2_s = constp.tile([128, H2], dt.float32)
            nc.sync.dma_start(out=b2_s[:], in_=b2r[:])
            b3_s = constp.tile([128, OUT_C], dt.float32)
            nc.sync.dma_start(out=b3_s[:], in_=b3r[:])
            iota_s = constp.tile([128, 128], dt.bfloat16)
            nc.sync.dma_start(out=iota_s[:], in_=iota[:])
            ident = constp.tile([128, 128], dt.float32)
            make_identity(nc, ident[:])
            zpad = constp.tile([128, TW3 - 1], dt.bfloat16)
            nc.vector.memset(zpad[:], 0.0)

            # ---- DRAM temporaries ----
            loc12 = dram.tile([nmax, TW], dt.bfloat16)
            adl12 = dram.tile([nmax, TW3], dt.bfloat16)
            loc3 = dram.tile([nmax, TW3], dt.bfloat16)
            adl3 = dram.tile([nmax, TW3], dt.bfloat16)

            # zero never-written pad columns once (NaN hygiene)
            for buf, c0 in ((loc12, GW), (adl12, HEADS), (loc3, GW3),
                            (adl3, 1)):
                w = buf.shape[1] - c0
                nc.sync.dma_start(
                    out=buf[:].rearrange("(t p) w -> p t w", p=128)[:, :, c0:],
                    in_=zpad[:, :w].unsqueeze(1).to_broadcast(
                        [128, ntiles, w]))

            def pack12(ps, local, adl, t):
                pk = epil.tile([128, GW], dt.bfloat16, tag="pack")
                nc.scalar.copy(out=pk[:], in_=ps[:, :GW])
                nc.sync.dma_start(out=local[t * 128:(t + 1) * 128, :GW],
                                  in_=pk[:])
                pa = epil.tile([128, HEADS], dt.bfloat16, tag="packa")
                nc.scalar.copy(out=pa[:], in_=ps[:, GW:GW + HEADS])
                nc.sync.dma_start(out=adl[t * 128:(t + 1) * 128, :HEADS],
                                  in_=pa[:])

            def pack3(ps, t):
                pk = epil.tile([128, GW3], dt.bfloat16, tag="pack")
                nc.scalar.copy(out=pk[:], in_=ps[:, :GW3])
                nc.sync.dma_start(out=loc3[t * 128:(t + 1) * 128, :GW3],
                                  in_=pk[:])
                pa = epil.tile([128, 1], dt.bfloat16, tag="packa")
                nc.scalar.copy(out=pa[:], in_=ps[:, GW3:GW3 + 1])
                nc.sync.dma_start(out=adl3[t * 128:(t + 1) * 128, :1],
                                  in_=pa[:])

            def h1_phase():
                for t in range(ntiles):
                    ps = psum_h.tile([128, PACK], dt.float32, tag="hps")
                    nc.tensor.matmul(
                        ps[:], lhsT=xT_s[:, t * 128:(t + 1) * 128],
                        rhs=rhs1_s[:], start=True, stop=True)
                    pack12(ps, loc12, adl12, t)

            def allgather(local, table):
                nc.gpsimd.collective_compute(
                    "AllGather", Alu.bypass,
                    replica_groups=[list(range(NCORES))],
                    ins=[local[:].opt()], outs=[table[:].opt()])

            def epilogue12(t, ps, rhs_next_s, b_s, layer):
                deneps = epil.tile([128, HEADS], dt.float32, tag="deneps")
                nc.vector.tensor_scalar_add(deneps[:], ps[:, H2:H2 + HEADS],
                                            1e-16)
                recip = epil.tile([128, HEADS], dt.float32, tag="recip")
                nc.vector.reciprocal(recip[:], deneps[:])
                act = epil.tile([128, H2], dt.float32, tag="act")
                nc.vector.tensor_tensor(
                    out=act[:].rearrange("p (c h) -> p c h", h=HEADS),
                    in0=ps[:, :H2].rearrange("p (c h) -> p c h", h=HEADS),
                    in1=recip[:].unsqueeze(1).to_broadcast([128, HID, HEADS]),
                    op=Alu.mult)
                nc.vector.tensor_add(out=act[:], in0=act[:], in1=b_s[:])
                nc.scalar.activation(out=act[:], in_=act[:], func=Act.Relu)
                w = PACK if layer == 1 else OUT_C + 2
                hps = psum_h.tile([128, PACK], dt.float32, tag="hps")
                for kc in range(2):
                    tp = psum_tp.tile([128, 128], dt.float32, tag="tp")
                    nc.tensor.transpose(
                        out=tp[:], in_=act[:, kc * 128:(kc + 1) * 128],
                        identity=ident[:])
                    aT = epil.tile([128, 128], dt.float32, tag="aT")
                    nc.scalar.copy(out=aT[:], in_=tp[:])
                    nc.tensor.matmul(
                        hps[:, :w], lhsT=aT[:],
                        rhs=rhs_next_s[:, kc * w:(kc + 1) * w],
                        start=(kc == 0), stop=(kc == 1))
                if layer == 1:
                    pack12(hps, loc12, adl12, t)
                else:
                    pack3(hps, t)

            def epilogue3(t, ps):
                deneps = epil.tile([128, 1], dt.float32, tag="deneps3")
                nc.vector.tensor_scalar_add(deneps[:], ps[:, OUT_C:OUT_C + 1],
                                            1e-16)
                recip = epil.tile([128, 1], dt.float32, tag="recip3")
                nc.vector.reciprocal(recip[:], deneps[:])
                o3 = epil.tile([128, OUT_C], dt.float32, tag="o3")
                nc.vector.tensor_scalar(
                    out=o3[:], in0=ps[:, :OUT_C], scalar1=recip[:, :1],
                    scalar2=None, op0=Alu.mult)
                nc.vector.tensor_add(out=o3[:], in0=o3[:], in1=b3_s[:])
                mneg = epil.tile([128, 1], dt.float32, tag="mneg")
                nc.vector.tensor_reduce(
                    out=mneg[:], in_=o3[:], axis=mybir.AxisListType.X,
                    op=Alu.max, negate=True)
                es = epil.tile([128, OUT_C], dt.float32, tag="es")
                ssum = epil.tile([128, 1], dt.float32, tag="ssum")
                nc.scalar.activation(out=es[:], in_=o3[:], func=Act.Exp,
                                     bias=mneg[:, :1], accum_out=ssum[:, :1])
                lse = epil.tile([128, 1], dt.float32, tag="lse")
                nc.scalar.activation(out=lse[:], in_=ssum[:], func=Act.Ln)
                fin = epil.tile([128, OUT_C], dt.float32, tag="fin")
                nc.vector.tensor_scalar(
                    out=fin[:], in0=o3[:], scalar1=mneg[:, :1],
                    scalar2=lse[:, :1], op0=Alu.add, op1=Alu.subtract)
                nc.sync.dma_start(out=out[t * 128:(t + 1) * 128, :], in_=fin[:])

            def aggregate(layer, table, adl, rhs_next_s, b_s):
                if layer == 3:
                    gw, nfeat, nh, tw = GW3, OUT_C, 1, TW3
                else:
                    gw, nfeat, nh, tw = GW, H2, HEADS, TW

                ps_cur = None
                for sup in range(nsup):
                    sidx = sbuf.tile([128, KSUP * 8], dt.int16, tag="sidx")
                    nc.sync.dma_start(
                        out=sidx[:],
                        in_=src_w[:, sup * KSUP * 8:(sup + 1) * KSUP * 8])
                    didx = sbuf.tile([128, KSUP * 8], dt.int16, tag="didx")
                    nc.sync.dma_start(
                        out=didx[:],
                        in_=adst_w[:, sup * KSUP * 8:(sup + 1) * KSUP * 8])
                    segt = sbuf.tile([128, KSUP], dt.float32, tag="segt")
                    nc.sync.dma_start(out=segt[:], in_=seg_in[sup])

                    gt = gbuf.tile([128, KSUP, tw], dt.bfloat16,
                                   tag=f"g{min(layer, 2)}")
                    for (a, b, tag) in runs_by_sup[sup]:
                        a0, b0 = a - sup * KSUP, b - sup * KSUP
                        nidx = (b - a) * 128
                        src_ap = (table[:half, :] if tag == 0
                                  else table[half:2 * half, :])
                        nc.gpsimd.dma_gather(
                            out_ap=gt[:, a0:b0, :], in_ap=src_ap,
                            idxs_ap=sidx[:, a0 * 8:b0 * 8],
                            num_idxs=nidx, num_idxs_reg=nidx, elem_size=tw,
                            queue_num=0)
                    dts = gbuf.tile([128, KSUP, TW3], dt.bfloat16, tag="dts")
                    nc.gpsimd.dma_gather(
                        out_ap=dts[:], in_ap=adl[:], idxs_ap=didx[:],
                        num_idxs=KSUP * 128, num_idxs_reg=KSUP * 128,
                        elem_size=TW3, queue_num=0)

                    wt = gbuf.tile([128, KSUP, nh], dt.bfloat16, tag="wt")
                    nc.vector.tensor_tensor(
                        out=wt[:], in0=gt[:, :, nfeat:nfeat + nh],
                        in1=dts[:, :, :nh], op=Alu.add)
                    lk = gbuf.tile([128, KSUP, nh], dt.bfloat16, tag="lk")
                    nc.vector.tensor_scalar_mul(lk[:], wt[:], NEG_SLOPE)
                    nc.vector.tensor_tensor(out=wt[:], in0=wt[:], in1=lk[:],
                                            op=Alu.max)
                    nc.scalar.activation(out=wt[:], in_=wt[:], func=Act.Exp)
                    if layer != 3:
                        nc.vector.tensor_tensor(
                            out=gt[:, :, :nfeat].rearrange(
                                "p k (c h) -> p k c h", h=HEADS),
                            in0=gt[:, :, :nfeat].rearrange(
                                "p k (c h) -> p k c h", h=HEADS),
                            in1=wt[:].unsqueeze(2).to_broadcast(
                                [128, KSUP, HID, HEADS]),
                            op=Alu.mult)
                    else:
                        nc.vector.tensor_tensor(
                            out=gt[:, :, :nfeat], in0=gt[:, :, :nfeat],
                            in1=wt[:].to_broadcast([128, KSUP, nfeat]),
                            op=Alu.mult)
                    nc.vector.tensor_copy(gt[:, :, nfeat:nfeat + nh], wt[:])

                    mt = mbuf.tile([128, KSUP * 128], dt.bfloat16, tag="mt")
                    for kk in range(KSUP):
                        q = sup * KSUP + kk
                        t = int(tile_of_chunk[q])
                        nc.vector.tensor_scalar(
                            out=mt[:, kk * 128:(kk + 1) * 128], in0=iota_s[:],
                            scalar1=segt[:, kk:kk + 1], scalar2=None,
                            op0=Alu.is_equal)
                        if q == first_chunk[t]:
                            ps_cur = psum_seg.tile([128, GW], dt.float32,
                                                   tag="segps")
                        nc.tensor.matmul(
                            ps_cur[:, :gw],
                            lhsT=mt[:, kk * 128:(kk + 1) * 128],
                            rhs=gt[:, kk, :gw],
                            start=(q == first_chunk[t]),
                            stop=(q == last_chunk[t]))
                        if q == last_chunk[t]:
                            if layer == 3:
                                epilogue3(t, ps_cur)
                            else:
                                epilogue12(t, ps_cur, rhs_next_s, b_s, layer)

            import os
            nphase = int(os.environ.get("GAT_PHASES", "3"))
            for _rep in range(repeat):
                tab1 = dram.tile([NCORES * nmax, TW], dt.bfloat16,
                                 addr_space="Shared", name=f"tab1_{_rep}")
                tab2 = dram.tile([NCORES * nmax, TW], dt.bfloat16,
                                 addr_space="Shared", name=f"tab2_{_rep}")
                tab3 = dram.tile([NCORES * nmax, TW3], dt.bfloat16,
                                 addr_space="Shared", name=f"tab3_{_rep}")
                h1_phase()
                allgather(loc12, tab1)
                if nphase >= 1:
                    aggregate(1, tab1, adl12, rhs2_s, b1_s)
                if nphase >= 2:
                    allgather(loc12, tab2)
                    aggregate(2, tab2, adl12, rhs3_s, b2_s)
                if nphase >= 3:
                    allgather(loc3, tab3)
                    aggregate(3, tab3, adl3, None, None)

    nc.compile()
    return nc


_CACHE = {}


def kernel(x, edge_index, W1, as1, ad1, b1, W2, as2, ad2, b2, W3, as3, ad3, b3,
           _repeat=1):
    from concourse.bass_utils import run_bass_kernel_spmd

    x = np.asarray(x, np.float32)
    edge_index = np.asarray(edge_index)
    g = _prep_graph(edge_index)
    rhs1, rhs2, rhs3, b1r, b2r, b3r = _prep_weights(
        W1, as1, ad1, b1, W2, as2, ad2, b2, W3, as3, ad3, b3)

    key = (hash(edge_index.tobytes()), _repeat)
    if key not in _CACHE:
        _CACHE[key] = _build_bass(g, repeat=_repeat)
    nc = _CACHE[key]

    npc, nmax = g["npc"], g["nmax"]
    iota = np.tile(np.arange(128, dtype=np.float32)[None, :],
                   (128, 1)).astype(_BF16)
    in_maps = []
    for k in range(NCORES):
        xT = np.zeros((IN_C, nmax), np.float32)
        xT[:, :npc] = x[k * npc:(k + 1) * npc].T
        in_maps.append({
            "xT": xT, "rhs1": rhs1, "rhs2": rhs2, "rhs3": rhs3,
            "b1r": b1r, "b2r": b2r, "b3r": b3r, "iota": iota,
            "src_w": g["src_w"][k], "adst_w": g["adst_w"][k],
            "seg": g["seg"][k],
        })

    res = run_bass_kernel_spmd(nc, in_maps, core_ids=list(range(NCORES)))
    outf = np.zeros((N, OUT_C), np.float32)
    for k in range(NCORES):
        outf[k * npc:(k + 1) * npc] = res.results[k]["out"][:npc]
    return outf



# revision 13
# speedup vs baseline: 1.8566x; 1.8566x over previous
"""3-layer GAT on 8 Trainium2 NeuronCores (Bass/Tile).

Edge-sharded by destination range:
  - Nodes split into 8 contiguous ranges (one per core); each core owns the
    softmax + aggregation for its destination nodes.
  - Per layer a packed per-node table [h | a_src] (c-major feature order) is
    computed locally and AllGathered (bf16, 768B rows); a_dst lives in a
    per-core local table (256B rows).
  - Edges (with self loops) are bucketed per core into 128-dst tiles x
    128-edge chunks; chunk structure (incl. lo/hi int16-index table halves)
    is made identical across cores so one SPMD instruction stream fits all.
  - Per 8-chunk super-batch the kernel dma_gathers source rows + dest
    attention rows, computes w = exp(leaky_relu(a_src+a_dst)) (softmax
    shift-invariance removes the segment-max pass at these value ranges),
    scales messages by w, and segment-sums with matmuls against one-hot
    membership matrices (tensor_scalar is_equal vs an iota tile), keeping
    numerator and denominator together in PSUM.  The per-tile epilogue
    divides, applies bias/relu, and feeds the next layer's matmul whose rhs
    [W | W@att_src | W@att_dst] also emits the next attention scores.
"""

import numpy as np
import ml_dtypes

N = 50000
E = 800000
IN_C = 128
HID = 32
OUT_C = 40
HEADS = 8
NEG_SLOPE = 0.2
NCORES = 8

_BF16 = ml_dtypes.bfloat16

KSUP = 8  # chunks per gather super-batch (1024 idx = dma_gather limit)


def _cmajor_perm(heads, ch):
    f_new = np.arange(heads * ch)
    return (f_new % heads) * ch + f_new // heads  # perm[new] = old


def _attn_cols(w, att):
    heads, ch = att.shape
    return np.einsum("khc,hc->kh", w.reshape(-1, heads, ch), att).astype(np.float32)


def _prep_weights(W1, as1, ad1, b1, W2, as2, ad2, b2, W3, as3, ad3, b3):
    W1 = np.asarray(W1, np.float32)
    W2 = np.asarray(W2, np.float32)
    W3 = np.asarray(W3, np.float32)
    perm = _cmajor_perm(HEADS, HID)

    rhs1 = np.concatenate(
        [W1[:, perm], _attn_cols(W1, np.asarray(as1, np.float32)),
         _attn_cols(W1, np.asarray(ad1, np.float32))], axis=1).astype(np.float32)
    W2r = W2[perm, :]
    rhs2 = np.concatenate(
        [W2r[:, perm], _attn_cols(W2r, np.asarray(as2, np.float32)),
         _attn_cols(W2r, np.asarray(ad2, np.float32))], axis=1).astype(np.float32)
    W3r = W3[perm, :]
    as3p = (W3r @ np.asarray(as3, np.float32)[0]).reshape(-1, 1)
    ad3p = (W3r @ np.asarray(ad3, np.float32)[0]).reshape(-1, 1)
    rhs3 = np.concatenate([W3r, as3p, ad3p], axis=1).astype(np.float32)

    def bcast(b):
        return np.tile(np.asarray(b, np.float32)[None, :], (128, 1))

    return (rhs1, rhs2, rhs3,
            bcast(np.asarray(b1, np.float32)[perm]),
            bcast(np.asarray(b2, np.float32)[perm]),
            bcast(np.asarray(b3, np.float32)))


def _prep_graph(edge_index):
    """Slot edges into the SPMD-uniform (tile, section, chunk) grid."""
    src = np.concatenate([edge_index[0], np.arange(N)]).astype(np.int64)
    dst = np.concatenate([edge_index[1], np.arange(N)]).astype(np.int64)

    npc = N // NCORES
    ntiles = (npc + 127) // 128
    nmax = ntiles * 128
    half = (NCORES // 2) * nmax

    core_of = dst // npc
    d_loc = dst - core_of * npc
    tile_of = d_loc // 128
    s_core = src // npc
    s_row = s_core * nmax + (src - s_core * npc)  # AG-table row of src
    is_hi = s_row >= half

    cnt = np.zeros((NCORES, ntiles, 2), np.int64)
    np.add.at(cnt, (core_of, tile_of, is_hi.astype(np.int64)), 1)
    sec_cpt = np.ceil(cnt / 128).astype(np.int64).max(axis=0)  # [ntiles, 2]
    sec_cpt[:, 0] = np.maximum(sec_cpt[:, 0], 1)

    total = int(sec_cpt.sum())
    pad = (-total) % KSUP
    sec_cpt[-1, 1] += pad
    total += pad
    nsup = total // KSUP

    tile_of_chunk = []
    tag_of_chunk = []
    for t in range(ntiles):
        tile_of_chunk += [t] * int(sec_cpt[t, 0] + sec_cpt[t, 1])
        tag_of_chunk += [0] * int(sec_cpt[t, 0]) + [1] * int(sec_cpt[t, 1])
    tile_of_chunk = np.array(tile_of_chunk)
    tag_of_chunk = np.array(tag_of_chunk)
    sec_base = np.zeros((ntiles, 2), np.int64)
    sec_base.ravel()[1:] = np.cumsum(sec_cpt.ravel())[:-1]

    src_w = np.zeros((NCORES, 128, total * 8), np.int16)
    adst_w = np.zeros((NCORES, 128, total * 8), np.int16)
    seg = np.full((NCORES, nsup, 128, KSUP), 255.0, np.float32)

    order = np.lexsort((src, is_hi, tile_of, core_of))
    src_o = s_row[order]
    dst_o = d_loc[order]
    core_o = core_of[order]
    tile_o = tile_of[order]
    hi_o = is_hi[order]

    for k in range(NCORES):
        m = core_o == k
        t = tile_o[m]
        hi = hi_o[m].astype(np.int64)
        sr = src_o[m] - hi * half
        dl = dst_o[m]
        key = t * 2 + hi
        cnts = np.bincount(key, minlength=ntiles * 2)
        st = np.zeros(ntiles * 2, np.int64)
        st[1:] = np.cumsum(cnts)[:-1]
        pos = np.arange(len(t)) - st[key]
        q = sec_base[t, hi] + pos // 128
        p = pos % 128
        col = q * 8 + p // 16
        row = p % 16
        for c in range(8):
            src_w[k, row + 16 * c, col] = sr
            adst_w[k, row + 16 * c, col] = dl
        seg[k, q // KSUP, p, q % KSUP] = (dl % 128).astype(np.float32)

    runs = []  # (sup, chunk_lo, chunk_hi, tag)
    for s in range(nsup):
        q0 = s * KSUP
        r0 = q0
        for q in range(q0 + 1, q0 + KSUP + 1):
            if q == q0 + KSUP or tag_of_chunk[q] != tag_of_chunk[r0]:
                runs.append((s, r0, q, int(tag_of_chunk[r0])))
                r0 = q

    return dict(
        src_w=src_w, adst_w=adst_w, seg=seg,
        tile_of_chunk=tile_of_chunk, runs=runs, nsup=nsup, total=total,
        ntiles=ntiles, nmax=nmax, npc=npc, half=half,
    )


def _build_bass(g, repeat=1, abl=()):
    abl = set(abl)
    import concourse.bacc as bacc
    import concourse.mybir as mybir
    import concourse.tile as tile
    from concourse.masks import make_identity

    dt = mybir.dt
    Alu = mybir.AluOpType
    Act = mybir.ActivationFunctionType

    ntiles, nmax, nsup, total = g["ntiles"], g["nmax"], g["nsup"], g["total"]
    half = g["half"]
    tile_of_chunk = g["tile_of_chunk"]
    H2 = HEADS * HID  # 256
    PACK = H2 + 2 * HEADS  # 272 psum width: h + a_src + a_dst
    TW = 384  # AG table row width (768B)
    TW3 = 128  # layer-3 / a_dst table row width (256B)
    GW = H2 + HEADS  # 264 useful gathered cols
    GW3 = OUT_C + 1  # 41

    first_chunk = {}
    last_chunk = {}
    for q, t in enumerate(tile_of_chunk):
        first_chunk.setdefault(int(t), q)
        last_chunk[int(t)] = q
    runs_by_sup = {}
    for (s, a, b, tag) in g["runs"]:
        runs_by_sup.setdefault(s, []).append((a, b, tag))

    nq = 2 if "q2" in abl else 1
    nc = bacc.Bacc("TRN2", target_bir_lowering=False, debug=False,
                   num_devices=NCORES, num_swdge_queues=nq)

    xT = nc.dram_tensor("xT", [IN_C, nmax], dt.float32, kind="ExternalInput")
    rhs1 = nc.dram_tensor("rhs1", [IN_C, PACK], dt.float32, kind="ExternalInput")
    rhs2 = nc.dram_tensor("rhs2", [H2, PACK], dt.float32, kind="ExternalInput")
    rhs3 = nc.dram_tensor("rhs3", [H2, OUT_C + 2], dt.float32,
                          kind="ExternalInput")
    b1r = nc.dram_tensor("b1r", [128, H2], dt.float32, kind="ExternalInput")
    b2r = nc.dram_tensor("b2r", [128, H2], dt.float32, kind="ExternalInput")
    b3r = nc.dram_tensor("b3r", [128, OUT_C], dt.float32, kind="ExternalInput")
    iota = nc.dram_tensor("iota", [128, 128], dt.bfloat16, kind="ExternalInput")
    src_w = nc.dram_tensor("src_w", [128, total * 8], dt.int16,
                           kind="ExternalInput")
    adst_w = nc.dram_tensor("adst_w", [128, total * 8], dt.int16,
                            kind="ExternalInput")
    seg_in = nc.dram_tensor("seg", [nsup, 128, KSUP], dt.float32,
                            kind="ExternalInput")
    out = nc.dram_tensor("out", [nmax, OUT_C], dt.float32,
                         kind="ExternalOutput")


    with tile.TileContext(nc) as tc:
        with (
            tc.tile_pool(name="const", bufs=1) as constp,
            tc.tile_pool(name="sbuf", bufs=4) as sbuf,
            tc.tile_pool(name="gbuf", bufs=4) as gbuf,
            tc.tile_pool(name="mbuf", bufs=4) as mbuf,
            tc.tile_pool(name="epil", bufs=2) as epil,
            tc.tile_pool(name="psum_seg", bufs=3, space="PSUM") as psum_seg,
            tc.tile_pool(name="psum_h", bufs=2, space="PSUM") as psum_h,
            tc.tile_pool(name="psum_tp", bufs=2, space="PSUM") as psum_tp,
            tc.tile_pool(name="dram", bufs=1, space="DRAM") as dram,
        ):
            # ---- constants ----
            xT_s = constp.tile([IN_C, nmax], dt.float32)
            nc.sync.dma_start(out=xT_s[:], in_=xT[:])
            rhs1_s = constp.tile([IN_C, PACK], dt.float32)
            nc.sync.dma_start(out=rhs1_s[:], in_=rhs1[:])
            rhs2_s = constp.tile([128, 2 * PACK], dt.float32)
            nc.sync.dma_start(
                out=rhs2_s[:].rearrange("p (k f) -> p k f", k=2),
                in_=rhs2[:].rearrange("(k p) f -> p k f", p=128))
            rhs3_s = constp.tile([128, 2 * (OUT_C + 2)], dt.float32)
            nc.sync.dma_start(
                out=rhs3_s[:].rearrange("p (k f) -> p k f", k=2),
                in_=rhs3[:].rearrange("(k p) f -> p k f", p=128))
            b1_s = constp.tile([128, H2], dt.float32)
            nc.sync.dma_start(out=b1_s[:], in_=b1r[:])
            b2_s = constp.tile([128, H2], dt.float32)
            nc.sync.dma_start(out=b2_s[:], in_=b2r[:])
            b3_s = constp.tile([128, OUT_C], dt.float32)
            nc.sync.dma_start(out=b3_s[:], in_=b3r[:])
            iota_s = constp.tile([128, 128], dt.bfloat16)
            nc.sync.dma_start(out=iota_s[:], in_=iota[:])
            ident = constp.tile([128, 128], dt.float32)
            make_identity(nc, ident[:])
            zpad = constp.tile([128, TW3 - 1], dt.bfloat16)
            nc.vector.memset(zpad[:], 0.0)

            # ---- DRAM temporaries (per-layer: no intra-phase overwrite
            # hazards between dst-gathers and epilogue pack writes) ----
            loc1 = dram.tile([nmax, TW], dt.bfloat16)
            adl1 = dram.tile([nmax, TW3], dt.bfloat16)
            loc2 = dram.tile([nmax, TW], dt.bfloat16)
            adl2 = dram.tile([nmax, TW3], dt.bfloat16)
            loc3 = dram.tile([nmax, TW3], dt.bfloat16)
            adl3 = dram.tile([nmax, TW3], dt.bfloat16)

            # zero never-written pad columns once (NaN hygiene)
            for buf, c0 in ((loc1, GW), (adl1, HEADS), (loc2, GW),
                            (adl2, HEADS), (loc3, GW3), (adl3, 1)):
                w = buf.shape[1] - c0
                nc.sync.dma_start(
                    out=buf[:].rearrange("(t p) w -> p t w", p=128)[:, :, c0:],
                    in_=zpad[:, :w].unsqueeze(1).to_broadcast(
                        [128, ntiles, w]))

            def pack12(ps, local, adl, t):
                pk = epil.tile([128, GW], dt.bfloat16, tag="pack")
                nc.scalar.copy(out=pk[:], in_=ps[:, :GW])
                nc.sync.dma_start(out=local[t * 128:(t + 1) * 128, :GW],
                                  in_=pk[:])
                pa = epil.tile([128, HEADS], dt.bfloat16, tag="packa")
                nc.scalar.copy(out=pa[:], in_=ps[:, GW:GW + HEADS])
                nc.sync.dma_start(out=adl[t * 128:(t + 1) * 128, :HEADS],
                                  in_=pa[:])

            def pack3(ps, t):
                pk = epil.tile([128, GW3], dt.bfloat16, tag="pack")
                nc.scalar.copy(out=pk[:], in_=ps[:, :GW3])
                nc.sync.dma_start(out=loc3[t * 128:(t + 1) * 128, :GW3],
                                  in_=pk[:])
                pa = epil.tile([128, 1], dt.bfloat16, tag="packa")
                nc.scalar.copy(out=pa[:], in_=ps[:, GW3:GW3 + 1])
                nc.sync.dma_start(out=adl3[t * 128:(t + 1) * 128, :1],
                                  in_=pa[:])

            def h1_phase():
                for t in range(ntiles):
                    ps = psum_h.tile([128, PACK], dt.float32, tag="hps")
                    nc.tensor.matmul(
                        ps[:], lhsT=xT_s[:, t * 128:(t + 1) * 128],
                        rhs=rhs1_s[:], start=True, stop=True)
                    pack12(ps, loc1, adl1, t)

            def allgather(local, table):
                if "noag" in abl:
                    nc.sync.dma_start(out=table[:local.shape[0]], in_=local[:])
                    return
                nc.gpsimd.collective_compute(
                    "AllGather", Alu.bypass,
                    replica_groups=[list(range(NCORES))],
                    ins=[local[:].opt()], outs=[table[:].opt()])

            def epilogue12(t, ps, rhs_next_s, b_s, layer):
                deneps = epil.tile([128, HEADS], dt.float32, tag="deneps")
                nc.vector.tensor_scalar_add(deneps[:], ps[:, H2:H2 + HEADS],
                                            1e-16)
                recip = epil.tile([128, HEADS], dt.float32, tag="recip")
                nc.vector.reciprocal(recip[:], deneps[:])
                act = epil.tile([128, H2], dt.float32, tag="act")
                nc.vector.tensor_tensor(
                    out=act[:].rearrange("p (c h) -> p c h", h=HEADS),
                    in0=ps[:, :H2].rearrange("p (c h) -> p c h", h=HEADS),
                    in1=recip[:].unsqueeze(1).to_broadcast([128, HID, HEADS]),
                    op=Alu.mult)
                nc.vector.tensor_add(out=act[:], in0=act[:], in1=b_s[:])
                nc.scalar.activation(out=act[:], in_=act[:], func=Act.Relu)
                w = PACK if layer == 1 else OUT_C + 2
                hps = psum_h.tile([128, PACK], dt.float32, tag="hps")
                for kc in range(2):
                    tp = psum_tp.tile([128, 128], dt.float32, tag="tp")
                    nc.tensor.transpose(
                        out=tp[:], in_=act[:, kc * 128:(kc + 1) * 128],
                        identity=ident[:])
                    aT = epil.tile([128, 128], dt.float32, tag="aT")
                    nc.scalar.copy(out=aT[:], in_=tp[:])
                    nc.tensor.matmul(
                        hps[:, :w], lhsT=aT[:],
                        rhs=rhs_next_s[:, kc * w:(kc + 1) * w],
                        start=(kc == 0), stop=(kc == 1))
                if layer == 1:
                    pack12(hps, loc2, adl2, t)
                else:
                    pack3(hps, t)

            def epilogue3(t, ps):
                deneps = epil.tile([128, 1], dt.float32, tag="deneps3")
                nc.vector.tensor_scalar_add(deneps[:], ps[:, OUT_C:OUT_C + 1],
                                            1e-16)
                recip = epil.tile([128, 1], dt.float32, tag="recip3")
                nc.vector.reciprocal(recip[:], deneps[:])
                o3 = epil.tile([128, OUT_C], dt.float32, tag="o3")
                nc.vector.tensor_scalar(
                    out=o3[:], in0=ps[:, :OUT_C], scalar1=recip[:, :1],
                    scalar2=None, op0=Alu.mult)
                nc.vector.tensor_add(out=o3[:], in0=o3[:], in1=b3_s[:])
                mneg = epil.tile([128, 1], dt.float32, tag="mneg")
                nc.vector.tensor_reduce(
                    out=mneg[:], in_=o3[:], axis=mybir.AxisListType.X,
                    op=Alu.max, negate=True)
                es = epil.tile([128, OUT_C], dt.float32, tag="es")
                ssum = epil.tile([128, 1], dt.float32, tag="ssum")
                nc.scalar.activation(out=es[:], in_=o3[:], func=Act.Exp,
                                     bias=mneg[:, :1], accum_out=ssum[:, :1])
                lse = epil.tile([128, 1], dt.float32, tag="lse")
                nc.scalar.activation(out=lse[:], in_=ssum[:], func=Act.Ln)
                fin = epil.tile([128, OUT_C], dt.float32, tag="fin")
                nc.vector.tensor_scalar(
                    out=fin[:], in0=o3[:], scalar1=mneg[:, :1],
                    scalar2=lse[:, :1], op0=Alu.add, op1=Alu.subtract)
                nc.sync.dma_start(out=out[t * 128:(t + 1) * 128, :], in_=fin[:])

            def aggregate(layer, table, adl, rhs_next_s, b_s):
                if layer == 3:
                    gw, nfeat, nh, tw = GW3, OUT_C, 1, TW3
                else:
                    gw, nfeat, nh, tw = GW, H2, HEADS, TW

                ps_cur = None
                for sup in range(nsup):
                    sidx = sbuf.tile([128, KSUP * 8], dt.int16, tag="sidx")
                    nc.sync.dma_start(
                        out=sidx[:],
                        in_=src_w[:, sup * KSUP * 8:(sup + 1) * KSUP * 8])
                    didx = sbuf.tile([128, KSUP * 8], dt.int16, tag="didx")
                    nc.sync.dma_start(
                        out=didx[:],
                        in_=adst_w[:, sup * KSUP * 8:(sup + 1) * KSUP * 8])
                    segt = sbuf.tile([128, KSUP], dt.float32, tag="segt")
                    nc.sync.dma_start(out=segt[:], in_=seg_in[sup])

                    gt = gbuf.tile([128, KSUP, tw], dt.bfloat16,
                                   tag=f"g{min(layer, 2)}")
                    for (a, b, tag) in runs_by_sup[sup]:
                        a0, b0 = a - sup * KSUP, b - sup * KSUP
                        nidx = (b - a) * 128
                        src_ap = (table[:half, :] if tag == 0
                                  else table[half:2 * half, :])
                        nc.gpsimd.dma_gather(
                            out_ap=gt[:, a0:b0, :], in_ap=src_ap,
                            idxs_ap=sidx[:, a0 * 8:b0 * 8],
                            num_idxs=nidx, num_idxs_reg=nidx, elem_size=tw,
                            queue_num=0)
                    if "2xsrc" in abl:
                        gt2 = gbuf.tile([128, KSUP, tw], dt.bfloat16,
                                        tag=f"g2x{min(layer, 2)}")
                        for (a, b, tag) in runs_by_sup[sup]:
                            a0, b0 = a - sup * KSUP, b - sup * KSUP
                            nidx = (b - a) * 128
                            src_ap = (table[:half, :] if tag == 0
                                      else table[half:2 * half, :])
                            nc.gpsimd.dma_gather(
                                out_ap=gt2[:, a0:b0, :], in_ap=src_ap,
                                idxs_ap=sidx[:, a0 * 8:b0 * 8],
                                num_idxs=nidx, num_idxs_reg=nidx, elem_size=tw,
                                queue_num=0)
                    if "nodst" not in abl:
                        dts = gbuf.tile([128, KSUP, TW3], dt.bfloat16,
                                        tag="dts")
                        nc.gpsimd.dma_gather(
                            out_ap=dts[:], in_ap=adl[:], idxs_ap=didx[:],
                            num_idxs=KSUP * 128, num_idxs_reg=KSUP * 128,
                            elem_size=TW3, queue_num=nq - 1)
                    if "2xdst" in abl:
                        dts2 = gbuf.tile([128, KSUP, TW3], dt.bfloat16,
                                         tag="dts2x")
                        nc.gpsimd.dma_gather(
                            out_ap=dts2[:], in_ap=adl[:], idxs_ap=didx[:],
                            num_idxs=KSUP * 128, num_idxs_reg=KSUP * 128,
                            elem_size=TW3, queue_num=0)

                    wt = gbuf.tile([128, KSUP, nh], dt.bfloat16, tag="wt")
                    if "novec" not in abl:
                        nc.vector.tensor_tensor(
                            out=wt[:], in0=gt[:, :, nfeat:nfeat + nh],
                            in1=(gt[:, :, nfeat:nfeat + nh] if "nodst" in abl
                                 else dts[:, :, :nh]), op=Alu.add)
                        lk = gbuf.tile([128, KSUP, nh], dt.bfloat16, tag="lk")
                        nc.vector.tensor_scalar_mul(lk[:], wt[:], NEG_SLOPE)
                        nc.vector.tensor_tensor(out=wt[:], in0=wt[:], in1=lk[:],
                                                op=Alu.max)
                        nc.scalar.activation(out=wt[:], in_=wt[:], func=Act.Exp)
                        if layer != 3:
                            nc.vector.tensor_tensor(
                                out=gt[:, :, :nfeat].rearrange(
                                    "p k (c h) -> p k c h", h=HEADS),
                                in0=gt[:, :, :nfeat].rearrange(
                                    "p k (c h) -> p k c h", h=HEADS),
                                in1=wt[:].unsqueeze(2).to_broadcast(
                                    [128, KSUP, HID, HEADS]),
                                op=Alu.mult)
                        else:
                            nc.vector.tensor_tensor(
                                out=gt[:, :, :nfeat], in0=gt[:, :, :nfeat],
                                in1=wt[:].to_broadcast([128, KSUP, nfeat]),
                                op=Alu.mult)
                        nc.vector.tensor_copy(gt[:, :, nfeat:nfeat + nh], wt[:])
                        if "2xvec" in abl:
                            gv = gbuf.tile([128, KSUP, tw], dt.bfloat16,
                                           tag=f"gv{min(layer, 2)}")
                            if layer != 3:
                                nc.vector.tensor_tensor(
                                    out=gv[:, :, :nfeat].rearrange(
                                        "p k (c h) -> p k c h", h=HEADS),
                                    in0=gt[:, :, :nfeat].rearrange(
                                        "p k (c h) -> p k c h", h=HEADS),
                                    in1=wt[:].unsqueeze(2).to_broadcast(
                                        [128, KSUP, HID, HEADS]),
                                    op=Alu.mult)
                            else:
                                nc.vector.tensor_tensor(
                                    out=gv[:, :, :nfeat], in0=gt[:, :, :nfeat],
                                    in1=wt[:].to_broadcast([128, KSUP, nfeat]),
                                    op=Alu.mult)

                    if "nomm" in abl:
                        continue
                    mt = mbuf.tile([128, KSUP * 128], dt.bfloat16, tag="mt")
                    for kk in range(KSUP):
                        q = sup * KSUP + kk
                        t = int(tile_of_chunk[q])
                        nc.vector.tensor_scalar(
                            out=mt[:, kk * 128:(kk + 1) * 128], in0=iota_s[:],
                            scalar1=segt[:, kk:kk + 1], scalar2=None,
                            op0=Alu.is_equal)
                        if q == first_chunk[t]:
                            ps_cur = psum_seg.tile([128, GW], dt.float32,
                                                   tag="segps")
                        nc.tensor.matmul(
                            ps_cur[:, :gw],
                            lhsT=mt[:, kk * 128:(kk + 1) * 128],
                            rhs=gt[:, kk, :gw],
                            start=(q == first_chunk[t]),
                            stop=(q == last_chunk[t]))
                        if "2xmm" in abl:
                            ps2 = psum_seg.tile([128, GW], dt.float32,
                                                tag="segps2x")
                            nc.tensor.matmul(
                                ps2[:, :gw],
                                lhsT=mt[:, kk * 128:(kk + 1) * 128],
                                rhs=gt[:, kk, :gw],
                                start=True, stop=True)
                        if q == last_chunk[t] and "noepi" not in abl:
                            if layer == 3:
                                epilogue3(t, ps_cur)
                            else:
                                epilogue12(t, ps_cur, rhs_next_s, b_s, layer)

            import os
            nphase = int(os.environ.get("GAT_PHASES", "3"))
            for _rep in range(repeat):
                tab1 = dram.tile([NCORES * nmax, TW], dt.bfloat16,
                                 addr_space="Shared", name=f"tab1_{_rep}")
                tab2 = dram.tile([NCORES * nmax, TW], dt.bfloat16,
                                 addr_space="Shared", name=f"tab2_{_rep}")
                tab3 = dram.tile([NCORES * nmax, TW3], dt.bfloat16,
                                 addr_space="Shared", name=f"tab3_{_rep}")
                h1_phase()
                allgather(loc1, tab1)
                if nphase >= 1:
                    aggregate(1, tab1, adl1, rhs2_s, b1_s)
                if nphase >= 2:
                    allgather(loc2, tab2)
                    aggregate(2, tab2, adl2, rhs3_s, b2_s)
                if nphase >= 3:
                    allgather(loc3, tab3)
                    aggregate(3, tab3, adl3, None, None)

    nc.compile()
    return nc


_CACHE = {}


def kernel(x, edge_index, W1, as1, ad1, b1, W2, as2, ad2, b2, W3, as3, ad3, b3,
           _repeat=1):
    from concourse.bass_utils import run_bass_kernel_spmd

    x = np.asarray(x, np.float32)
    edge_index = np.asarray(edge_index)
    g = _prep_graph(edge_index)
    rhs1, rhs2, rhs3, b1r, b2r, b3r = _prep_weights(
        W1, as1, ad1, b1, W2, as2, ad2, b2, W3, as3, ad3, b3)

    key = (hash(edge_index.tobytes()), _repeat)
    if key not in _CACHE:
        _CACHE[key] = _build_bass(g, repeat=_repeat)
    nc = _CACHE[key]

    npc, nmax = g["npc"], g["nmax"]
    iota = np.tile(np.arange(128, dtype=np.float32)[None, :],
                   (128, 1)).astype(_BF16)
    in_maps = []
    for k in range(NCORES):
        xT = np.zeros((IN_C, nmax), np.float32)
        xT[:, :npc] = x[k * npc:(k + 1) * npc].T
        in_maps.append({
            "xT": xT, "rhs1": rhs1, "rhs2": rhs2, "rhs3": rhs3,
            "b1r": b1r, "b2r": b2r, "b3r": b3r, "iota": iota,
            "src_w": g["src_w"][k], "adst_w": g["adst_w"][k],
            "seg": g["seg"][k],
        })

    res = run_bass_kernel_spmd(nc, in_maps, core_ids=list(range(NCORES)))
    outf = np.zeros((N, OUT_C), np.float32)
    for k in range(NCORES):
        outf[k * npc:(k + 1) * npc] = res.results[k]["out"][:npc]
    return outf



# revision 18
# speedup vs baseline: 2.2174x; 1.1943x over previous
"""3-layer GAT on 8 Trainium2 NeuronCores (Bass/Tile).

Edge-sharded by destination range:
  - Nodes split into 8 contiguous ranges (one per core); each core owns the
    softmax + aggregation for its destination nodes.
  - Per layer a packed per-node table [h | a_src] (c-major feature order) is
    computed locally and AllGathered (bf16, 768B rows); a_dst lives in a
    per-core local table (256B rows).
  - Edges (with self loops) are bucketed per core into 128-dst tiles x
    128-edge chunks; chunk structure (incl. lo/hi int16-index table halves)
    is made identical across cores so one SPMD instruction stream fits all.
  - Per 8-chunk super-batch the kernel dma_gathers source rows + dest
    attention rows, computes w = exp(leaky_relu(a_src+a_dst)) (softmax
    shift-invariance removes the segment-max pass at these value ranges),
    scales messages by w, and segment-sums with matmuls against one-hot
    membership matrices (tensor_scalar is_equal vs an iota tile), keeping
    numerator and denominator together in PSUM.  The per-tile epilogue
    divides, applies bias/relu, and feeds the next layer's matmul whose rhs
    [W | W@att_src | W@att_dst] also emits the next attention scores.
"""

import numpy as np
import ml_dtypes

N = 50000
E = 800000
IN_C = 128
HID = 32
OUT_C = 40
HEADS = 8
NEG_SLOPE = 0.2
NCORES = 8

_BF16 = ml_dtypes.bfloat16

KSUP = 8  # chunks per gather super-batch (1024 idx = dma_gather limit)


def _cmajor_perm(heads, ch):
    f_new = np.arange(heads * ch)
    return (f_new % heads) * ch + f_new // heads  # perm[new] = old


def _attn_cols(w, att):
    heads, ch = att.shape
    return np.einsum("khc,hc->kh", w.reshape(-1, heads, ch), att).astype(np.float32)


def _prep_weights(W1, as1, ad1, b1, W2, as2, ad2, b2, W3, as3, ad3, b3):
    W1 = np.asarray(W1, np.float32)
    W2 = np.asarray(W2, np.float32)
    W3 = np.asarray(W3, np.float32)
    perm = _cmajor_perm(HEADS, HID)

    rhs1 = np.concatenate(
        [W1[:, perm], _attn_cols(W1, np.asarray(as1, np.float32)),
         _attn_cols(W1, np.asarray(ad1, np.float32))], axis=1).astype(np.float32)
    W2r = W2[perm, :]
    rhs2 = np.concatenate(
        [W2r[:, perm], _attn_cols(W2r, np.asarray(as2, np.float32)),
         _attn_cols(W2r, np.asarray(ad2, np.float32))], axis=1).astype(np.float32)
    W3r = W3[perm, :]
    as3p = (W3r @ np.asarray(as3, np.float32)[0]).reshape(-1, 1)
    ad3p = (W3r @ np.asarray(ad3, np.float32)[0]).reshape(-1, 1)
    rhs3 = np.concatenate([W3r, as3p, ad3p], axis=1).astype(np.float32)

    def bcast(b):
        return np.tile(np.asarray(b, np.float32)[None, :], (128, 1))

    return (rhs1, rhs2, rhs3,
            bcast(np.asarray(b1, np.float32)[perm]),
            bcast(np.asarray(b2, np.float32)[perm]),
            bcast(np.asarray(b3, np.float32)))


def _prep_graph(edge_index):
    """Slot edges into the SPMD-uniform (tile, section, chunk) grid."""
    src = np.concatenate([edge_index[0], np.arange(N)]).astype(np.int64)
    dst = np.concatenate([edge_index[1], np.arange(N)]).astype(np.int64)

    npc = N // NCORES
    ntiles = (npc + 127) // 128
    nmax = ntiles * 128
    half = (NCORES // 2) * nmax

    core_of = dst // npc
    d_loc = dst - core_of * npc
    tile_of = d_loc // 128
    s_core = src // npc
    s_row = s_core * nmax + (src - s_core * npc)  # AG-table row of src
    is_hi = s_row >= half

    cnt = np.zeros((NCORES, ntiles, 2), np.int64)
    np.add.at(cnt, (core_of, tile_of, is_hi.astype(np.int64)), 1)
    sec_cpt = np.ceil(cnt / 128).astype(np.int64).max(axis=0)  # [ntiles, 2]
    sec_cpt[:, 0] = np.maximum(sec_cpt[:, 0], 1)

    total = int(sec_cpt.sum())
    pad = (-total) % KSUP
    sec_cpt[-1, 1] += pad
    total += pad
    nsup = total // KSUP

    tile_of_chunk = []
    tag_of_chunk = []
    for t in range(ntiles):
        tile_of_chunk += [t] * int(sec_cpt[t, 0] + sec_cpt[t, 1])
        tag_of_chunk += [0] * int(sec_cpt[t, 0]) + [1] * int(sec_cpt[t, 1])
    tile_of_chunk = np.array(tile_of_chunk)
    tag_of_chunk = np.array(tag_of_chunk)
    sec_base = np.zeros((ntiles, 2), np.int64)
    sec_base.ravel()[1:] = np.cumsum(sec_cpt.ravel())[:-1]

    src_w = np.zeros((NCORES, 128, total * 8), np.int16)
    seg = np.full((NCORES, nsup, 128, KSUP), 255.0, np.float32)
    sta = np.zeros((NCORES, nsup, 128, KSUP), np.float32)
    fin = np.zeros((NCORES, nsup, 128, KSUP), np.float32)

    # slots sorted by dst within each (core, tile, hi) section so each
    # chunk's slot->dst map is a set of contiguous runs ([sta, fin) per dst)
    order = np.lexsort((dst, is_hi, tile_of, core_of))
    src_o = s_row[order]
    dst_o = d_loc[order]
    core_o = core_of[order]
    tile_o = tile_of[order]
    hi_o = is_hi[order]

    for k in range(NCORES):
        m = core_o == k
        t = tile_o[m]
        hi = hi_o[m].astype(np.int64)
        sr = src_o[m] - hi * half
        dl = dst_o[m]
        key = t * 2 + hi
        cnts = np.bincount(key, minlength=ntiles * 2)
        st = np.zeros(ntiles * 2, np.int64)
        st[1:] = np.cumsum(cnts)[:-1]
        pos = np.arange(len(t)) - st[key]
        q = sec_base[t, hi] + pos // 128
        p = pos % 128
        col = q * 8 + p // 16
        row = p % 16
        for c in range(8):
            src_w[k, row + 16 * c, col] = sr
        seg[k, q // KSUP, p, q % KSUP] = (dl % 128).astype(np.float32)

    dgrid = np.arange(128)
    for k in range(NCORES):
        for s in range(nsup):
            for kk in range(KSUP):
                dlm = seg[k, s, :, kk]
                sta[k, s, :, kk] = np.searchsorted(dlm, dgrid, side="left")
                fin[k, s, :, kk] = np.searchsorted(dlm, dgrid, side="right")

    runs = []  # (sup, chunk_lo, chunk_hi, tag)
    for s in range(nsup):
        q0 = s * KSUP
        r0 = q0
        for q in range(q0 + 1, q0 + KSUP + 1):
            if q == q0 + KSUP or tag_of_chunk[q] != tag_of_chunk[r0]:
                runs.append((s, r0, q, int(tag_of_chunk[r0])))
                r0 = q

    return dict(
        src_w=src_w, seg=seg, sta=sta, fin=fin,
        tile_of_chunk=tile_of_chunk, runs=runs, nsup=nsup, total=total,
        ntiles=ntiles, nmax=nmax, npc=npc, half=half,
    )


def _build_bass(g, repeat=1, abl=()):
    abl = set(abl)
    import concourse.bacc as bacc
    import concourse.mybir as mybir
    import concourse.tile as tile
    from concourse.masks import make_identity

    dt = mybir.dt
    Alu = mybir.AluOpType
    Act = mybir.ActivationFunctionType

    ntiles, nmax, nsup, total = g["ntiles"], g["nmax"], g["nsup"], g["total"]
    half = g["half"]
    tile_of_chunk = g["tile_of_chunk"]
    H2 = HEADS * HID  # 256
    PACK = H2 + 2 * HEADS  # 272 psum width: h + a_src + a_dst
    TW = 384  # AG table row width (768B)
    TW3 = 128  # layer-3 / a_dst table row width (256B)
    GW = H2 + HEADS  # 264 useful gathered cols
    GW3 = OUT_C + 1  # 41

    first_chunk = {}
    last_chunk = {}
    for q, t in enumerate(tile_of_chunk):
        first_chunk.setdefault(int(t), q)
        last_chunk[int(t)] = q
    runs_by_sup = {}
    for (s, a, b, tag) in g["runs"]:
        runs_by_sup.setdefault(s, []).append((a, b, tag))

    nq = 2 if "q2" in abl else 1
    nc = bacc.Bacc("TRN2", target_bir_lowering=False, debug=False,
                   num_devices=NCORES, num_swdge_queues=nq)

    xT = nc.dram_tensor("xT", [IN_C, nmax], dt.float32, kind="ExternalInput")
    rhs1 = nc.dram_tensor("rhs1", [IN_C, PACK], dt.float32, kind="ExternalInput")
    rhs2 = nc.dram_tensor("rhs2", [H2, PACK], dt.float32, kind="ExternalInput")
    rhs3 = nc.dram_tensor("rhs3", [H2, OUT_C + 2], dt.float32,
                          kind="ExternalInput")
    b1r = nc.dram_tensor("b1r", [128, H2], dt.float32, kind="ExternalInput")
    b2r = nc.dram_tensor("b2r", [128, H2], dt.float32, kind="ExternalInput")
    b3r = nc.dram_tensor("b3r", [128, OUT_C], dt.float32, kind="ExternalInput")
    iota = nc.dram_tensor("iota", [128, 128], dt.bfloat16, kind="ExternalInput")
    src_w = nc.dram_tensor("src_w", [128, total * 8], dt.int16,
                           kind="ExternalInput")
    seg_in = nc.dram_tensor("seg", [nsup, 128, KSUP], dt.bfloat16,
                            kind="ExternalInput")
    sta_in = nc.dram_tensor("sta", [nsup, 128, KSUP], dt.bfloat16,
                            kind="ExternalInput")
    fin_in = nc.dram_tensor("fin", [nsup, 128, KSUP], dt.bfloat16,
                            kind="ExternalInput")
    out = nc.dram_tensor("out", [nmax, OUT_C], dt.float32,
                         kind="ExternalOutput")


    with tile.TileContext(nc) as tc:
        with (
            tc.tile_pool(name="const", bufs=1) as constp,
            tc.tile_pool(name="sbuf", bufs=4) as sbuf,
            tc.tile_pool(name="gbuf", bufs=4) as gbuf,
            tc.tile_pool(name="mbuf", bufs=4) as mbuf,
            tc.tile_pool(name="epil", bufs=2) as epil,
            tc.tile_pool(name="psum_seg", bufs=2, space="PSUM") as psum_seg,
            tc.tile_pool(name="psum_sl", bufs=2, space="PSUM") as psum_sl,
            tc.tile_pool(name="psum_h", bufs=2, space="PSUM") as psum_h,
            tc.tile_pool(name="psum_tp", bufs=2, space="PSUM") as psum_tp,
            tc.tile_pool(name="dram", bufs=1, space="DRAM") as dram,
        ):
            # ---- constants ----
            xT_s = constp.tile([IN_C, nmax], dt.float32)
            nc.sync.dma_start(out=xT_s[:], in_=xT[:])
            rhs1_s = constp.tile([IN_C, PACK], dt.float32)
            nc.sync.dma_start(out=rhs1_s[:], in_=rhs1[:])
            rhs2_s = constp.tile([128, 2 * PACK], dt.float32)
            nc.sync.dma_start(
                out=rhs2_s[:].rearrange("p (k f) -> p k f", k=2),
                in_=rhs2[:].rearrange("(k p) f -> p k f", p=128))
            rhs3_s = constp.tile([128, 2 * (OUT_C + 2)], dt.float32)
            nc.sync.dma_start(
                out=rhs3_s[:].rearrange("p (k f) -> p k f", k=2),
                in_=rhs3[:].rearrange("(k p) f -> p k f", p=128))
            b1_s = constp.tile([128, H2], dt.float32)
            nc.sync.dma_start(out=b1_s[:], in_=b1r[:])
            b2_s = constp.tile([128, H2], dt.float32)
            nc.sync.dma_start(out=b2_s[:], in_=b2r[:])
            b3_s = constp.tile([128, OUT_C], dt.float32)
            nc.sync.dma_start(out=b3_s[:], in_=b3r[:])
            iota_s = constp.tile([128, 128], dt.bfloat16)
            nc.sync.dma_start(out=iota_s[:], in_=iota[:])
            ident = constp.tile([128, 128], dt.float32)
            make_identity(nc, ident[:])
            zpad = constp.tile([128, TW3 - 1], dt.bfloat16)
            nc.vector.memset(zpad[:], 0.0)

            # ---- DRAM temporaries (per-layer: no intra-phase overwrite
            # hazards between dst-gathers and epilogue pack writes) ----
            loc1 = dram.tile([nmax, TW], dt.bfloat16)
            adl1 = dram.tile([128, ntiles * HEADS], dt.bfloat16)
            loc2 = dram.tile([nmax, TW], dt.bfloat16)
            adl2 = dram.tile([128, ntiles * HEADS], dt.bfloat16)
            loc3 = dram.tile([nmax, TW3], dt.bfloat16)
            adl3 = dram.tile([128, ntiles], dt.bfloat16)

            # zero never-written pad columns once (NaN hygiene)
            for buf, c0 in ((loc1, GW), (loc2, GW), (loc3, GW3)):
                w = buf.shape[1] - c0
                nc.sync.dma_start(
                    out=buf[:].rearrange("(t p) w -> p t w", p=128)[:, :, c0:],
                    in_=zpad[:, :w].unsqueeze(1).to_broadcast(
                        [128, ntiles, w]))

            def pack12(ps, local, adl, t):
                pk = epil.tile([128, GW], dt.bfloat16, tag="pack")
                nc.scalar.copy(out=pk[:], in_=ps[:, :GW])
                nc.sync.dma_start(out=local[t * 128:(t + 1) * 128, :GW],
                                  in_=pk[:])
                pa = epil.tile([128, HEADS], dt.bfloat16, tag="packa")
                nc.scalar.copy(out=pa[:], in_=ps[:, GW:GW + HEADS])
                nc.sync.dma_start(out=adl[:, t * HEADS:(t + 1) * HEADS],
                                  in_=pa[:])

            def pack3(ps, t):
                pk = epil.tile([128, GW3], dt.bfloat16, tag="pack")
                nc.scalar.copy(out=pk[:], in_=ps[:, :GW3])
                nc.sync.dma_start(out=loc3[t * 128:(t + 1) * 128, :GW3],
                                  in_=pk[:])
                pa = epil.tile([128, 1], dt.bfloat16, tag="packa")
                nc.scalar.copy(out=pa[:], in_=ps[:, GW3:GW3 + 1])
                nc.sync.dma_start(out=adl3[:, t:t + 1], in_=pa[:])

            def h1_phase():
                for t in range(ntiles):
                    ps = psum_h.tile([128, PACK], dt.float32, tag="hps")
                    nc.tensor.matmul(
                        ps[:], lhsT=xT_s[:, t * 128:(t + 1) * 128],
                        rhs=rhs1_s[:], start=True, stop=True)
                    pack12(ps, loc1, adl1, t)

            def allgather(local, table):
                if "noag" in abl:
                    nc.sync.dma_start(out=table[:local.shape[0]], in_=local[:])
                    return
                nc.gpsimd.collective_compute(
                    "AllGather", Alu.bypass,
                    replica_groups=[list(range(NCORES))],
                    ins=[local[:].opt()], outs=[table[:].opt()])

            def epilogue12(t, ps, rhs_next_s, b_s, layer):
                deneps = epil.tile([128, HEADS], dt.float32, tag="deneps")
                nc.vector.tensor_scalar_add(deneps[:], ps[:, H2:H2 + HEADS],
                                            1e-16)
                recip = epil.tile([128, HEADS], dt.float32, tag="recip")
                nc.vector.reciprocal(recip[:], deneps[:])
                act = epil.tile([128, H2], dt.float32, tag="act")
                nc.vector.tensor_tensor(
                    out=act[:].rearrange("p (c h) -> p c h", h=HEADS),
                    in0=ps[:, :H2].rearrange("p (c h) -> p c h", h=HEADS),
                    in1=recip[:].unsqueeze(1).to_broadcast([128, HID, HEADS]),
                    op=Alu.mult)
                nc.vector.tensor_add(out=act[:], in0=act[:], in1=b_s[:])
                nc.scalar.activation(out=act[:], in_=act[:], func=Act.Relu)
                w = PACK if layer == 1 else OUT_C + 2
                hps = psum_h.tile([128, PACK], dt.float32, tag="hps")
                for kc in range(2):
                    tp = psum_tp.tile([128, 128], dt.float32, tag="tp")
                    nc.tensor.transpose(
                        out=tp[:], in_=act[:, kc * 128:(kc + 1) * 128],
                        identity=ident[:])
                    aT = epil.tile([128, 128], dt.float32, tag="aT")
                    nc.scalar.copy(out=aT[:], in_=tp[:])
                    nc.tensor.matmul(
                        hps[:, :w], lhsT=aT[:],
                        rhs=rhs_next_s[:, kc * w:(kc + 1) * w],
                        start=(kc == 0), stop=(kc == 1))
                if layer == 1:
                    pack12(hps, loc2, adl2, t)
                else:
                    pack3(hps, t)

            def epilogue3(t, ps):
                deneps = epil.tile([128, 1], dt.float32, tag="deneps3")
                nc.vector.tensor_scalar_add(deneps[:], ps[:, OUT_C:OUT_C + 1],
                                            1e-16)
                recip = epil.tile([128, 1], dt.float32, tag="recip3")
                nc.vector.reciprocal(recip[:], deneps[:])
                o3 = epil.tile([128, OUT_C], dt.float32, tag="o3")
                nc.vector.tensor_scalar(
                    out=o3[:], in0=ps[:, :OUT_C], scalar1=recip[:, :1],
                    scalar2=None, op0=Alu.mult)
                nc.vector.tensor_add(out=o3[:], in0=o3[:], in1=b3_s[:])
                mneg = epil.tile([128, 1], dt.float32, tag="mneg")
                nc.vector.tensor_reduce(
                    out=mneg[:], in_=o3[:], axis=mybir.AxisListType.X,
                    op=Alu.max, negate=True)
                es = epil.tile([128, OUT_C], dt.float32, tag="es")
                ssum = epil.tile([128, 1], dt.float32, tag="ssum")
                nc.scalar.activation(out=es[:], in_=o3[:], func=Act.Exp,
                                     bias=mneg[:, :1], accum_out=ssum[:, :1])
                lse = epil.tile([128, 1], dt.float32, tag="lse")
                nc.scalar.activation(out=lse[:], in_=ssum[:], func=Act.Ln)
                fin = epil.tile([128, OUT_C], dt.float32, tag="fin")
                nc.vector.tensor_scalar(
                    out=fin[:], in0=o3[:], scalar1=mneg[:, :1],
                    scalar2=lse[:, :1], op0=Alu.add, op1=Alu.subtract)
                nc.sync.dma_start(out=out[t * 128:(t + 1) * 128, :], in_=fin[:])

            def aggregate(layer, table, adl, rhs_next_s, b_s):
                if layer == 3:
                    gw, nfeat, nh, tw = GW3, OUT_C, 1, TW3
                else:
                    gw, nfeat, nh, tw = GW, H2, HEADS, TW

                adl_s = sbuf.tile([128, ntiles * nh], dt.bfloat16,
                                  tag=f"adls{layer}")
                nc.sync.dma_start(out=adl_s[:], in_=adl[:])

                ps_cur = None
                for sup in range(nsup):
                    sidx = sbuf.tile([128, KSUP * 8], dt.int16, tag="sidx")
                    nc.sync.dma_start(
                        out=sidx[:],
                        in_=src_w[:, sup * KSUP * 8:(sup + 1) * KSUP * 8])
                    segt = sbuf.tile([128, KSUP], dt.bfloat16, tag="segt")
                    nc.sync.dma_start(out=segt[:], in_=seg_in[sup])
                    sta_s = sbuf.tile([128, KSUP], dt.bfloat16, tag="sta")
                    nc.sync.dma_start(out=sta_s[:], in_=sta_in[sup])
                    fin_s = sbuf.tile([128, KSUP], dt.bfloat16, tag="fin")
                    nc.sync.dma_start(out=fin_s[:], in_=fin_in[sup])

                    gt = gbuf.tile([128, KSUP, tw], dt.bfloat16,
                                   tag=f"g{min(layer, 2)}")
                    for (a, b, tag) in runs_by_sup[sup]:
                        a0, b0 = a - sup * KSUP, b - sup * KSUP
                        nidx = (b - a) * 128
                        src_ap = (table[:half, :] if tag == 0
                                  else table[half:2 * half, :])
                        nc.gpsimd.dma_gather(
                            out_ap=gt[:, a0:b0, :], in_ap=src_ap,
                            idxs_ap=sidx[:, a0 * 8:b0 * 8],
                            num_idxs=nidx, num_idxs_reg=nidx, elem_size=tw,
                            queue_num=0)

                    # transposed one-hot [dst, slot] per chunk from the
                    # dst-sorted slot ranges: mtT = (s >= sta) & (s < fin)
                    iotaB = iota_s[:].unsqueeze(1).to_broadcast(
                        [128, KSUP, 128])
                    tgA = gbuf.tile([128, KSUP, 128], dt.bfloat16, tag="tgA")
                    nc.vector.tensor_tensor(
                        out=tgA[:], in0=iotaB,
                        in1=sta_s[:].unsqueeze(2).to_broadcast(
                            [128, KSUP, 128]), op=Alu.is_ge)
                    tgB = gbuf.tile([128, KSUP, 128], dt.bfloat16, tag="tgB")
                    nc.vector.tensor_tensor(
                        out=tgB[:], in0=iotaB,
                        in1=fin_s[:].unsqueeze(2).to_broadcast(
                            [128, KSUP, 128]), op=Alu.is_lt)
                    mtT = gbuf.tile([128, KSUP, 128], dt.bfloat16, tag="mtT")
                    nc.vector.tensor_tensor(out=mtT[:], in0=tgA[:],
                                            in1=tgB[:], op=Alu.mult)

                    # expand per-dst attention to slots: ps_sl[s, h]
                    ps_sl = psum_sl.tile([128, KSUP * nh], dt.float32,
                                         tag="slps")
                    for kk in range(KSUP):
                        t = int(tile_of_chunk[sup * KSUP + kk])
                        nc.tensor.matmul(
                            ps_sl[:, kk * nh:(kk + 1) * nh],
                            lhsT=mtT[:, kk, :],
                            rhs=adl_s[:, t * nh:(t + 1) * nh],
                            start=True, stop=True)
                    sl_s = gbuf.tile([128, KSUP, nh], dt.bfloat16, tag="sls")
                    nc.scalar.copy(
                        out=sl_s[:],
                        in_=ps_sl[:].rearrange("p (k h) -> p k h", k=KSUP))

                    wt = gbuf.tile([128, KSUP, nh], dt.bfloat16, tag="wt")
                    if "novec" not in abl:
                        nc.vector.tensor_tensor(
                            out=wt[:], in0=gt[:, :, nfeat:nfeat + nh],
                            in1=sl_s[:], op=Alu.add)
                        lk = gbuf.tile([128, KSUP, nh], dt.bfloat16, tag="lk")
                        nc.vector.tensor_scalar_mul(lk[:], wt[:], NEG_SLOPE)
                        nc.vector.tensor_tensor(out=wt[:], in0=wt[:], in1=lk[:],
                                                op=Alu.max)
                        nc.scalar.activation(out=wt[:], in_=wt[:], func=Act.Exp)
                        if layer != 3:
                            nc.vector.tensor_tensor(
                                out=gt[:, :, :nfeat].rearrange(
                                    "p k (c h) -> p k c h", h=HEADS),
                                in0=gt[:, :, :nfeat].rearrange(
                                    "p k (c h) -> p k c h", h=HEADS),
                                in1=wt[:].unsqueeze(2).to_broadcast(
                                    [128, KSUP, HID, HEADS]),
                                op=Alu.mult)
                        else:
                            nc.vector.tensor_tensor(
                                out=gt[:, :, :nfeat], in0=gt[:, :, :nfeat],
                                in1=wt[:].to_broadcast([128, KSUP, nfeat]),
                                op=Alu.mult)
                        nc.vector.tensor_copy(gt[:, :, nfeat:nfeat + nh], wt[:])
                        if "2xvec" in abl:
                            gv = gbuf.tile([128, KSUP, tw], dt.bfloat16,
                                           tag=f"gv{min(layer, 2)}")
                            if layer != 3:
                                nc.vector.tensor_tensor(
                                    out=gv[:, :, :nfeat].rearrange(
                                        "p k (c h) -> p k c h", h=HEADS),
                                    in0=gt[:, :, :nfeat].rearrange(
                                        "p k (c h) -> p k c h", h=HEADS),
                                    in1=wt[:].unsqueeze(2).to_broadcast(
                                        [128, KSUP, HID, HEADS]),
                                    op=Alu.mult)
                            else:
                                nc.vector.tensor_tensor(
                                    out=gv[:, :, :nfeat], in0=gt[:, :, :nfeat],
                                    in1=wt[:].to_broadcast([128, KSUP, nfeat]),
                                    op=Alu.mult)

                    if "nomm" in abl:
                        continue
                    mt = mbuf.tile([128, KSUP, 128], dt.bfloat16, tag="mt")
                    nc.vector.tensor_tensor(
                        out=mt[:], in0=iotaB,
                        in1=segt[:].unsqueeze(2).to_broadcast(
                            [128, KSUP, 128]), op=Alu.is_equal)
                    for kk in range(KSUP):
                        q = sup * KSUP + kk
                        t = int(tile_of_chunk[q])
                        if q == first_chunk[t]:
                            ps_cur = psum_seg.tile([128, GW], dt.float32,
                                                   tag="segps")
                        nc.tensor.matmul(
                            ps_cur[:, :gw],
                            lhsT=mt[:, kk, :],
                            rhs=gt[:, kk, :gw],
                            start=(q == first_chunk[t]),
                            stop=(q == last_chunk[t]))
                        if "2xmm" in abl:
                            ps2 = psum_seg.tile([128, GW], dt.float32,
                                                tag="segps2x")
                            nc.tensor.matmul(
                                ps2[:, :gw],
                                lhsT=mt[:, kk, :],
                                rhs=gt[:, kk, :gw],
                                start=True, stop=True)
                        if q == last_chunk[t] and "noepi" not in abl:
                            if layer == 3:
                                epilogue3(t, ps_cur)
                            else:
                                epilogue12(t, ps_cur, rhs_next_s, b_s, layer)

            import os
            nphase = int(os.environ.get("GAT_PHASES", "3"))
            for _rep in range(repeat):
                tab1 = dram.tile([NCORES * nmax, TW], dt.bfloat16,
                                 addr_space="Shared", name=f"tab1_{_rep}")
                tab2 = dram.tile([NCORES * nmax, TW], dt.bfloat16,
                                 addr_space="Shared", name=f"tab2_{_rep}")
                tab3 = dram.tile([NCORES * nmax, TW3], dt.bfloat16,
                                 addr_space="Shared", name=f"tab3_{_rep}")
                h1_phase()
                allgather(loc1, tab1)
                if nphase >= 1:
                    aggregate(1, tab1, adl1, rhs2_s, b1_s)
                if nphase >= 2:
                    allgather(loc2, tab2)
                    aggregate(2, tab2, adl2, rhs3_s, b2_s)
                if nphase >= 3:
                    allgather(loc3, tab3)
                    aggregate(3, tab3, adl3, None, None)

    nc.compile()
    return nc


_CACHE = {}


def kernel(x, edge_index, W1, as1, ad1, b1, W2, as2, ad2, b2, W3, as3, ad3, b3,
           _repeat=1):
    from concourse.bass_utils import run_bass_kernel_spmd

    x = np.asarray(x, np.float32)
    edge_index = np.asarray(edge_index)
    g = _prep_graph(edge_index)
    rhs1, rhs2, rhs3, b1r, b2r, b3r = _prep_weights(
        W1, as1, ad1, b1, W2, as2, ad2, b2, W3, as3, ad3, b3)

    key = (hash(edge_index.tobytes()), _repeat)
    if key not in _CACHE:
        _CACHE[key] = _build_bass(g, repeat=_repeat)
    nc = _CACHE[key]

    npc, nmax = g["npc"], g["nmax"]
    iota = np.tile(np.arange(128, dtype=np.float32)[None, :],
                   (128, 1)).astype(_BF16)
    in_maps = []
    for k in range(NCORES):
        xT = np.zeros((IN_C, nmax), np.float32)
        xT[:, :npc] = x[k * npc:(k + 1) * npc].T
        in_maps.append({
            "xT": xT, "rhs1": rhs1, "rhs2": rhs2, "rhs3": rhs3,
            "b1r": b1r, "b2r": b2r, "b3r": b3r, "iota": iota,
            "src_w": g["src_w"][k], "seg": g["seg"][k].astype(_BF16),
            "sta": g["sta"][k].astype(_BF16), "fin": g["fin"][k].astype(_BF16),
        })

    res = run_bass_kernel_spmd(nc, in_maps, core_ids=list(range(NCORES)))
    outf = np.zeros((N, OUT_C), np.float32)
    for k in range(NCORES):
        outf[k * npc:(k + 1) * npc] = res.results[k]["out"][:npc]
    return outf



# revision 24
# speedup vs baseline: 11.5754x; 5.2202x over previous
"""3-layer GAT on 8 Trainium2 NeuronCores (Bass/Tile).

Edge-sharded by destination range:
  - Nodes split into 8 contiguous ranges (one per core); each core owns the
    softmax + aggregation for its destination nodes.
  - Per layer a packed per-node table [h | a_src] (c-major feature order) is
    computed locally and AllGathered (bf16, 768B rows); a_dst lives in a
    per-core local table (256B rows).
  - Edges (with self loops) are bucketed per core into 128-dst tiles x
    128-edge chunks; chunk structure (incl. lo/hi int16-index table halves)
    is made identical across cores so one SPMD instruction stream fits all.
  - Per 8-chunk super-batch the kernel dma_gathers source rows + dest
    attention rows, computes w = exp(leaky_relu(a_src+a_dst)) (softmax
    shift-invariance removes the segment-max pass at these value ranges),
    scales messages by w, and segment-sums with matmuls against one-hot
    membership matrices (tensor_scalar is_equal vs an iota tile), keeping
    numerator and denominator together in PSUM.  The per-tile epilogue
    divides, applies bias/relu, and feeds the next layer's matmul whose rhs
    [W | W@att_src | W@att_dst] also emits the next attention scores.
"""

import numpy as np
import ml_dtypes

N = 50000
E = 800000
IN_C = 128
HID = 32
OUT_C = 40
HEADS = 8
NEG_SLOPE = 0.2
NCORES = 8

_BF16 = ml_dtypes.bfloat16

KSUP = 8  # chunks per gather super-batch (1024 idx = dma_gather limit)


def _cmajor_perm(heads, ch):
    f_new = np.arange(heads * ch)
    return (f_new % heads) * ch + f_new // heads  # perm[new] = old


def _attn_cols(w, att):
    heads, ch = att.shape
    return np.einsum("khc,hc->kh", w.reshape(-1, heads, ch), att).astype(np.float32)


def _prep_weights(W1, as1, ad1, b1, W2, as2, ad2, b2, W3, as3, ad3, b3):
    W1 = np.asarray(W1, np.float32)
    W2 = np.asarray(W2, np.float32)
    W3 = np.asarray(W3, np.float32)
    perm = _cmajor_perm(HEADS, HID)

    rhs1 = np.concatenate(
        [W1[:, perm], _attn_cols(W1, np.asarray(as1, np.float32)),
         _attn_cols(W1, np.asarray(ad1, np.float32))], axis=1).astype(np.float32)
    W2r = W2[perm, :]
    rhs2 = np.concatenate(
        [W2r[:, perm], _attn_cols(W2r, np.asarray(as2, np.float32)),
         _attn_cols(W2r, np.asarray(ad2, np.float32))], axis=1).astype(np.float32)
    W3r = W3[perm, :]
    as3p = (W3r @ np.asarray(as3, np.float32)[0]).reshape(-1, 1)
    ad3p = (W3r @ np.asarray(ad3, np.float32)[0]).reshape(-1, 1)
    rhs3 = np.concatenate([W3r, as3p, ad3p], axis=1).astype(np.float32)

    def bcast(b):
        return np.tile(np.asarray(b, np.float32)[None, :], (128, 1))

    return (rhs1, rhs2, rhs3,
            bcast(np.asarray(b1, np.float32)[perm]),
            bcast(np.asarray(b2, np.float32)[perm]),
            bcast(np.asarray(b3, np.float32)))


def _prep_graph(edge_index):
    """Slot edges into the SPMD-uniform (tile, section, chunk) grid."""
    src = np.concatenate([edge_index[0], np.arange(N)]).astype(np.int64)
    dst = np.concatenate([edge_index[1], np.arange(N)]).astype(np.int64)

    npc = N // NCORES
    ntiles = (npc + 127) // 128
    nmax = ntiles * 128
    half = (NCORES // 2) * nmax

    core_of = dst // npc
    d_loc = dst - core_of * npc
    tile_of = d_loc // 128
    s_core = src // npc
    s_row = s_core * nmax + (src - s_core * npc)  # AG-table row of src
    is_hi = s_row >= half

    cnt = np.zeros((NCORES, ntiles, 2), np.int64)
    np.add.at(cnt, (core_of, tile_of, is_hi.astype(np.int64)), 1)
    sec_cpt = np.ceil(cnt / 128).astype(np.int64).max(axis=0)  # [ntiles, 2]
    sec_cpt[:, 0] = np.maximum(sec_cpt[:, 0], 1)

    total = int(sec_cpt.sum())
    pad = (-total) % KSUP
    sec_cpt[-1, 1] += pad
    total += pad
    nsup = total // KSUP

    tile_of_chunk = []
    tag_of_chunk = []
    for t in range(ntiles):
        tile_of_chunk += [t] * int(sec_cpt[t, 0] + sec_cpt[t, 1])
        tag_of_chunk += [0] * int(sec_cpt[t, 0]) + [1] * int(sec_cpt[t, 1])
    tile_of_chunk = np.array(tile_of_chunk)
    tag_of_chunk = np.array(tag_of_chunk)
    sec_base = np.zeros((ntiles, 2), np.int64)
    sec_base.ravel()[1:] = np.cumsum(sec_cpt.ravel())[:-1]

    src_w = np.zeros((NCORES, 128, total * 8), np.int16)
    seg = np.full((NCORES, nsup, 128, KSUP), 255.0, np.float32)

    order = np.lexsort((dst, is_hi, tile_of, core_of))
    src_o = s_row[order]
    dst_o = d_loc[order]
    core_o = core_of[order]
    tile_o = tile_of[order]
    hi_o = is_hi[order]

    for k in range(NCORES):
        m = core_o == k
        t = tile_o[m]
        hi = hi_o[m].astype(np.int64)
        sr = src_o[m] - hi * half
        dl = dst_o[m]
        key = t * 2 + hi
        cnts = np.bincount(key, minlength=ntiles * 2)
        st = np.zeros(ntiles * 2, np.int64)
        st[1:] = np.cumsum(cnts)[:-1]
        pos = np.arange(len(t)) - st[key]
        q = sec_base[t, hi] + pos // 128
        p = pos % 128
        col = q * 8 + p // 16
        row = p % 16
        for c in range(8):
            src_w[k, row + 16 * c, col] = sr
        seg[k, q // KSUP, p, q % KSUP] = (dl % 128).astype(np.float32)

    # prematerialized transposed one-hot [dst, slot] per chunk (kk-major)
    # and fused per-sup [sidx | seg] metadata rows
    mtt = np.zeros((NCORES, nsup, 128, KSUP * 128), _BF16)
    meta = np.zeros((NCORES, nsup, 128, KSUP * 8 + KSUP), np.int16)
    for k in range(NCORES):
        eq = (seg[k].transpose(0, 2, 1)[:, None, :, :]
              == np.arange(128)[None, :, None, None])
        # eq: [nsup, 128 d, KSUP, 128 s]
        mtt[k] = eq.reshape(nsup, 128, KSUP * 128).astype(_BF16)
        meta[k, :, :, :KSUP * 8] = src_w[k].T.reshape(total // KSUP,
                                                      KSUP * 8, 128
                                                      ).transpose(0, 2, 1)
        meta[k, :, :, KSUP * 8:] = (
            seg[k].astype(_BF16).view(np.int16).reshape(nsup, 128, KSUP))

    runs = []  # (sup, chunk_lo, chunk_hi, tag)
    for s in range(nsup):
        q0 = s * KSUP
        r0 = q0
        for q in range(q0 + 1, q0 + KSUP + 1):
            if q == q0 + KSUP or tag_of_chunk[q] != tag_of_chunk[r0]:
                runs.append((s, r0, q, int(tag_of_chunk[r0])))
                r0 = q

    return dict(
        mtt=mtt, meta=meta,
        tile_of_chunk=tile_of_chunk, runs=runs, nsup=nsup, total=total,
        ntiles=ntiles, nmax=nmax, npc=npc, half=half,
    )


def _build_bass(g, repeat=1, abl=()):
    abl = set(abl)
    import concourse.bacc as bacc
    import concourse.mybir as mybir
    import concourse.tile as tile
    from concourse.masks import make_identity

    dt = mybir.dt
    Alu = mybir.AluOpType
    Act = mybir.ActivationFunctionType

    ntiles, nmax, nsup, total = g["ntiles"], g["nmax"], g["nsup"], g["total"]
    half = g["half"]
    tile_of_chunk = g["tile_of_chunk"]
    H2 = HEADS * HID  # 256
    PACK = H2 + 2 * HEADS  # 272 psum width: h + a_src + a_dst
    TW = 384  # AG table row width (768B)
    TW3 = 128  # layer-3 / a_dst table row width (256B)
    GW = H2 + HEADS  # 264 useful gathered cols
    GW3 = OUT_C + 1  # 41

    first_chunk = {}
    last_chunk = {}
    for q, t in enumerate(tile_of_chunk):
        first_chunk.setdefault(int(t), q)
        last_chunk[int(t)] = q
    runs_by_sup = {}
    for (s, a, b, tag) in g["runs"]:
        runs_by_sup.setdefault(s, []).append((a, b, tag))

    nq = 2 if "q2" in abl else 1
    nc = bacc.Bacc("TRN2", target_bir_lowering=False, debug=False,
                   num_devices=NCORES, num_swdge_queues=nq)

    xT = nc.dram_tensor("xT", [IN_C, nmax], dt.float32, kind="ExternalInput")
    rhs1 = nc.dram_tensor("rhs1", [IN_C, PACK], dt.float32, kind="ExternalInput")
    rhs2 = nc.dram_tensor("rhs2", [H2, PACK], dt.float32, kind="ExternalInput")
    rhs3 = nc.dram_tensor("rhs3", [H2, OUT_C + 2], dt.float32,
                          kind="ExternalInput")
    b1r = nc.dram_tensor("b1r", [128, H2], dt.float32, kind="ExternalInput")
    b2r = nc.dram_tensor("b2r", [128, H2], dt.float32, kind="ExternalInput")
    b3r = nc.dram_tensor("b3r", [128, OUT_C], dt.float32, kind="ExternalInput")
    iota = nc.dram_tensor("iota", [128, 128], dt.bfloat16, kind="ExternalInput")
    meta_in = nc.dram_tensor("meta", [nsup, 128, KSUP * 8 + KSUP], dt.int16,
                             kind="ExternalInput")
    mtt_in = nc.dram_tensor("mtt", [nsup, 128, KSUP * 128], dt.bfloat16,
                            kind="ExternalInput")
    out = nc.dram_tensor("out", [nmax, OUT_C], dt.float32,
                         kind="ExternalOutput")


    with tile.TileContext(nc) as tc:
        with (
            tc.tile_pool(name="const", bufs=1) as constp,
            tc.tile_pool(name="sbuf", bufs=4) as sbuf,
            tc.tile_pool(name="gbuf", bufs=4) as gbuf,
            tc.tile_pool(name="mbuf", bufs=4) as mbuf,
            tc.tile_pool(name="epil", bufs=2) as epil,
            tc.tile_pool(name="psum_seg", bufs=2, space="PSUM") as psum_seg,
            tc.tile_pool(name="psum_sl", bufs=2, space="PSUM") as psum_sl,
            tc.tile_pool(name="psum_h", bufs=2, space="PSUM") as psum_h,
            tc.tile_pool(name="psum_tp", bufs=2, space="PSUM") as psum_tp,
            tc.tile_pool(name="dram", bufs=1, space="DRAM") as dram,
        ):
            # ---- constants ----
            xT_s = constp.tile([IN_C, nmax], dt.float32)
            nc.sync.dma_start(out=xT_s[:], in_=xT[:])
            rhs1_s = constp.tile([IN_C, PACK], dt.float32)
            nc.sync.dma_start(out=rhs1_s[:], in_=rhs1[:])
            rhs2_s = constp.tile([128, 2 * PACK], dt.float32)
            nc.sync.dma_start(
                out=rhs2_s[:].rearrange("p (k f) -> p k f", k=2),
                in_=rhs2[:].rearrange("(k p) f -> p k f", p=128))
            rhs3_s = constp.tile([128, 2 * (OUT_C + 2)], dt.float32)
            nc.sync.dma_start(
                out=rhs3_s[:].rearrange("p (k f) -> p k f", k=2),
                in_=rhs3[:].rearrange("(k p) f -> p k f", p=128))
            b1_s = constp.tile([128, H2], dt.float32)
            nc.sync.dma_start(out=b1_s[:], in_=b1r[:])
            b2_s = constp.tile([128, H2], dt.float32)
            nc.sync.dma_start(out=b2_s[:], in_=b2r[:])
            b3_s = constp.tile([128, OUT_C], dt.float32)
            nc.sync.dma_start(out=b3_s[:], in_=b3r[:])
            iota_s = constp.tile([128, 128], dt.bfloat16)
            nc.sync.dma_start(out=iota_s[:], in_=iota[:])
            ident = constp.tile([128, 128], dt.float32)
            make_identity(nc, ident[:])
            zpad = constp.tile([128, TW3 - 1], dt.bfloat16)
            nc.vector.memset(zpad[:], 0.0)

            # ---- DRAM temporaries (per-layer: no intra-phase overwrite
            # hazards between dst-gathers and epilogue pack writes) ----
            loc1 = dram.tile([nmax, TW], dt.bfloat16)
            adl1 = dram.tile([128, ntiles * HEADS], dt.bfloat16)
            loc2 = dram.tile([nmax, TW], dt.bfloat16)
            adl2 = dram.tile([128, ntiles * HEADS], dt.bfloat16)
            loc3 = dram.tile([nmax, TW3], dt.bfloat16)
            adl3 = dram.tile([128, ntiles], dt.bfloat16)

            # zero never-written pad columns once (NaN hygiene)
            for buf, c0 in ((loc1, GW), (loc2, GW), (loc3, GW3)):
                w = buf.shape[1] - c0
                nc.sync.dma_start(
                    out=buf[:].rearrange("(t p) w -> p t w", p=128)[:, :, c0:],
                    in_=zpad[:, :w].unsqueeze(1).to_broadcast(
                        [128, ntiles, w]))

            def pack12(ps, local, adl, t):
                pk = epil.tile([128, GW], dt.bfloat16, tag="pack")
                nc.scalar.copy(out=pk[:], in_=ps[:, :GW])
                nc.sync.dma_start(out=local[t * 128:(t + 1) * 128, :GW],
                                  in_=pk[:])
                pa = epil.tile([128, HEADS], dt.bfloat16, tag="packa")
                nc.scalar.copy(out=pa[:], in_=ps[:, GW:GW + HEADS])
                nc.sync.dma_start(out=adl[:, t * HEADS:(t + 1) * HEADS],
                                  in_=pa[:])

            def pack3(ps, t):
                pk = epil.tile([128, GW3], dt.bfloat16, tag="pack")
                nc.scalar.copy(out=pk[:], in_=ps[:, :GW3])
                nc.sync.dma_start(out=loc3[t * 128:(t + 1) * 128, :GW3],
                                  in_=pk[:])
                pa = epil.tile([128, 1], dt.bfloat16, tag="packa")
                nc.scalar.copy(out=pa[:], in_=ps[:, GW3:GW3 + 1])
                nc.sync.dma_start(out=adl3[:, t:t + 1], in_=pa[:])

            def h1_phase():
                for t in range(ntiles):
                    ps = psum_h.tile([128, PACK], dt.float32, tag="hps")
                    nc.tensor.matmul(
                        ps[:], lhsT=xT_s[:, t * 128:(t + 1) * 128],
                        rhs=rhs1_s[:], start=True, stop=True)
                    pack12(ps, loc1, adl1, t)

            def allgather(local, table):
                if "noag" in abl:
                    nc.sync.dma_start(out=table[:local.shape[0]], in_=local[:])
                    return
                nc.gpsimd.collective_compute(
                    "AllGather", Alu.bypass,
                    replica_groups=[list(range(NCORES))],
                    ins=[local[:].opt()], outs=[table[:].opt()])

            def epilogue12(t, ps, rhs_next_s, b_s, layer):
                deneps = epil.tile([128, HEADS], dt.float32, tag="deneps")
                nc.vector.tensor_scalar_add(deneps[:], ps[:, H2:H2 + HEADS],
                                            1e-16)
                recip = epil.tile([128, HEADS], dt.float32, tag="recip")
                nc.vector.reciprocal(recip[:], deneps[:])
                act = epil.tile([128, H2], dt.float32, tag="act")
                nc.vector.tensor_tensor(
                    out=act[:].rearrange("p (c h) -> p c h", h=HEADS),
                    in0=ps[:, :H2].rearrange("p (c h) -> p c h", h=HEADS),
                    in1=recip[:].unsqueeze(1).to_broadcast([128, HID, HEADS]),
                    op=Alu.mult)
                nc.vector.tensor_add(out=act[:], in0=act[:], in1=b_s[:])
                nc.scalar.activation(out=act[:], in_=act[:], func=Act.Relu)
                w = PACK if layer == 1 else OUT_C + 2
                hps = psum_h.tile([128, PACK], dt.float32, tag="hps")
                for kc in range(2):
                    tp = psum_tp.tile([128, 128], dt.float32, tag="tp")
                    nc.tensor.transpose(
                        out=tp[:], in_=act[:, kc * 128:(kc + 1) * 128],
                        identity=ident[:])
                    aT = epil.tile([128, 128], dt.float32, tag="aT")
                    nc.scalar.copy(out=aT[:], in_=tp[:])
                    nc.tensor.matmul(
                        hps[:, :w], lhsT=aT[:],
                        rhs=rhs_next_s[:, kc * w:(kc + 1) * w],
                        start=(kc == 0), stop=(kc == 1))
                if layer == 1:
                    pack12(hps, loc2, adl2, t)
                else:
                    pack3(hps, t)

            def epilogue3(t, ps):
                deneps = epil.tile([128, 1], dt.float32, tag="deneps3")
                nc.vector.tensor_scalar_add(deneps[:], ps[:, OUT_C:OUT_C + 1],
                                            1e-16)
                recip = epil.tile([128, 1], dt.float32, tag="recip3")
                nc.vector.reciprocal(recip[:], deneps[:])
                o3 = epil.tile([128, OUT_C], dt.float32, tag="o3")
                nc.vector.tensor_scalar(
                    out=o3[:], in0=ps[:, :OUT_C], scalar1=recip[:, :1],
                    scalar2=None, op0=Alu.mult)
                nc.vector.tensor_add(out=o3[:], in0=o3[:], in1=b3_s[:])
                mneg = epil.tile([128, 1], dt.float32, tag="mneg")
                nc.vector.tensor_reduce(
                    out=mneg[:], in_=o3[:], axis=mybir.AxisListType.X,
                    op=Alu.max, negate=True)
                es = epil.tile([128, OUT_C], dt.float32, tag="es")
                ssum = epil.tile([128, 1], dt.float32, tag="ssum")
                nc.scalar.activation(out=es[:], in_=o3[:], func=Act.Exp,
                                     bias=mneg[:, :1], accum_out=ssum[:, :1])
                lse = epil.tile([128, 1], dt.float32, tag="lse")
                nc.scalar.activation(out=lse[:], in_=ssum[:], func=Act.Ln)
                fin = epil.tile([128, OUT_C], dt.float32, tag="fin")
                nc.vector.tensor_scalar(
                    out=fin[:], in0=o3[:], scalar1=mneg[:, :1],
                    scalar2=lse[:, :1], op0=Alu.add, op1=Alu.subtract)
                nc.sync.dma_start(out=out[t * 128:(t + 1) * 128, :], in_=fin[:])

            def aggregate(layer, table, adl, rhs_next_s, b_s):
                if layer == 3:
                    gw, nfeat, nh, tw = GW3, OUT_C, 1, TW3
                else:
                    gw, nfeat, nh, tw = GW, H2, HEADS, TW

                adl_s = sbuf.tile([128, ntiles * nh], dt.bfloat16,
                                  tag=f"adls{layer}")
                nc.sync.dma_start(out=adl_s[:], in_=adl[:])

                ps_cur = None
                for sup in range(nsup):
                    mets = sbuf.tile([128, KSUP * 8 + KSUP], dt.int16,
                                     tag="mets")
                    nc.sync.dma_start(out=mets[:], in_=meta_in[sup])
                    mtT = gbuf.tile([128, KSUP, 128], dt.bfloat16, tag="mtT")
                    nc.sync.dma_start(
                        out=mtT[:],
                        in_=mtt_in[sup].rearrange("p (k s) -> p k s", k=KSUP))

                    gt = gbuf.tile([128, KSUP, tw], dt.bfloat16,
                                   tag=f"g{min(layer, 2)}")
                    for (a, b, tag) in runs_by_sup[sup]:
                        a0, b0 = a - sup * KSUP, b - sup * KSUP
                        nidx = (b - a) * 128
                        src_ap = (table[:half, :] if tag == 0
                                  else table[half:2 * half, :])
                        nc.gpsimd.dma_gather(
                            out_ap=gt[:, a0:b0, :], in_ap=src_ap,
                            idxs_ap=mets[:, a0 * 8:b0 * 8],
                            num_idxs=nidx, num_idxs_reg=nidx, elem_size=tw,
                            queue_num=0)

                    # expand per-dst attention to slots: ps_sl[s, h]
                    ps_sl = psum_sl.tile([128, KSUP * nh], dt.float32,
                                         tag="slps")
                    for kk in range(KSUP):
                        t = int(tile_of_chunk[sup * KSUP + kk])
                        nc.tensor.matmul(
                            ps_sl[:, kk * nh:(kk + 1) * nh],
                            lhsT=mtT[:, kk, :],
                            rhs=adl_s[:, t * nh:(t + 1) * nh],
                            start=True, stop=True)
                    sl_s = gbuf.tile([128, KSUP, nh], dt.bfloat16, tag="sls")
                    nc.scalar.copy(
                        out=sl_s[:],
                        in_=ps_sl[:].rearrange("p (k h) -> p k h", k=KSUP))

                    wt = gbuf.tile([128, KSUP, nh], dt.bfloat16, tag="wt")
                    if "novec" not in abl:
                        nc.vector.tensor_tensor(
                            out=wt[:], in0=gt[:, :, nfeat:nfeat + nh],
                            in1=sl_s[:], op=Alu.add)
                        nc.scalar.activation(out=wt[:], in_=wt[:],
                                             func=Act.Lrelu, alpha=NEG_SLOPE)
                        nc.scalar.activation(out=wt[:], in_=wt[:], func=Act.Exp)
                        if layer != 3:
                            nc.vector.tensor_tensor(
                                out=gt[:, :, :nfeat].rearrange(
                                    "p k (c h) -> p k c h", h=HEADS),
                                in0=gt[:, :, :nfeat].rearrange(
                                    "p k (c h) -> p k c h", h=HEADS),
                                in1=wt[:].unsqueeze(2).to_broadcast(
                                    [128, KSUP, HID, HEADS]),
                                op=Alu.mult)
                        else:
                            nc.vector.tensor_tensor(
                                out=gt[:, :, :nfeat], in0=gt[:, :, :nfeat],
                                in1=wt[:].to_broadcast([128, KSUP, nfeat]),
                                op=Alu.mult)
                        nc.vector.tensor_copy(gt[:, :, nfeat:nfeat + nh], wt[:])
                        if "2xvec" in abl:
                            gv = gbuf.tile([128, KSUP, tw], dt.bfloat16,
                                           tag=f"gv{min(layer, 2)}")
                            if layer != 3:
                                nc.vector.tensor_tensor(
                                    out=gv[:, :, :nfeat].rearrange(
                                        "p k (c h) -> p k c h", h=HEADS),
                                    in0=gt[:, :, :nfeat].rearrange(
                                        "p k (c h) -> p k c h", h=HEADS),
                                    in1=wt[:].unsqueeze(2).to_broadcast(
                                        [128, KSUP, HID, HEADS]),
                                    op=Alu.mult)
                            else:
                                nc.vector.tensor_tensor(
                                    out=gv[:, :, :nfeat], in0=gt[:, :, :nfeat],
                                    in1=wt[:].to_broadcast([128, KSUP, nfeat]),
                                    op=Alu.mult)

                    if "nomm" in abl:
                        continue
                    mt = mbuf.tile([128, KSUP, 128], dt.bfloat16, tag="mt")
                    nc.vector.tensor_tensor(
                        out=mt[:],
                        in0=iota_s[:].unsqueeze(1).to_broadcast(
                            [128, KSUP, 128]),
                        in1=mets[:, KSUP * 8:].bitcast(
                            dt.bfloat16).unsqueeze(2).to_broadcast(
                            [128, KSUP, 128]), op=Alu.is_equal)
                    for kk in range(KSUP):
                        q = sup * KSUP + kk
                        t = int(tile_of_chunk[q])
                        if q == first_chunk[t]:
                            ps_cur = psum_seg.tile([128, GW], dt.float32,
                                                   tag="segps")
                        nc.tensor.matmul(
                            ps_cur[:, :gw],
                            lhsT=mt[:, kk, :],
                            rhs=gt[:, kk, :gw],
                            start=(q == first_chunk[t]),
                            stop=(q == last_chunk[t]))
                        if "2xmm" in abl:
                            ps2 = psum_seg.tile([128, GW], dt.float32,
                                                tag="segps2x")
                            nc.tensor.matmul(
                                ps2[:, :gw],
                                lhsT=mt[:, kk, :],
                                rhs=gt[:, kk, :gw],
                                start=True, stop=True)
                        if q == last_chunk[t] and "noepi" not in abl:
                            if layer == 3:
                                epilogue3(t, ps_cur)
                            else:
                                epilogue12(t, ps_cur, rhs_next_s, b_s, layer)

            import os
            nphase = int(os.environ.get("GAT_PHASES", "3"))
            for _rep in range(repeat):
                tab1 = dram.tile([NCORES * nmax, TW], dt.bfloat16,
                                 addr_space="Shared", name=f"tab1_{_rep}")
                tab2 = dram.tile([NCORES * nmax, TW], dt.bfloat16,
                                 addr_space="Shared", name=f"tab2_{_rep}")
                tab3 = dram.tile([NCORES * nmax, TW3], dt.bfloat16,
                                 addr_space="Shared", name=f"tab3_{_rep}")
                h1_phase()
                allgather(loc1, tab1)
                if nphase >= 1:
                    aggregate(1, tab1, adl1, rhs2_s, b1_s)
                if nphase >= 2:
                    allgather(loc2, tab2)
                    aggregate(2, tab2, adl2, rhs3_s, b2_s)
                if nphase >= 3:
                    allgather(loc3, tab3)
                    aggregate(3, tab3, adl3, None, None)

    nc.compile()
    return nc


_CACHE = {}


def kernel(x, edge_index, W1, as1, ad1, b1, W2, as2, ad2, b2, W3, as3, ad3, b3,
           _repeat=1):
    from concourse.bass_utils import run_bass_kernel_spmd

    x = np.asarray(x, np.float32)
    edge_index = np.asarray(edge_index)
    g = _prep_graph(edge_index)
    rhs1, rhs2, rhs3, b1r, b2r, b3r = _prep_weights(
        W1, as1, ad1, b1, W2, as2, ad2, b2, W3, as3, ad3, b3)

    key = (hash(edge_index.tobytes()), _repeat)
    if key not in _CACHE:
        _CACHE[key] = _build_bass(g, repeat=_repeat)
    nc = _CACHE[key]

    npc, nmax = g["npc"], g["nmax"]
    iota = np.tile(np.arange(128, dtype=np.float32)[None, :],
                   (128, 1)).astype(_BF16)
    in_maps = []
    for k in range(NCORES):
        xT = np.zeros((IN_C, nmax), np.float32)
        xT[:, :npc] = x[k * npc:(k + 1) * npc].T
        in_maps.append({
            "xT": xT, "rhs1": rhs1, "rhs2": rhs2, "rhs3": rhs3,
            "b1r": b1r, "b2r": b2r, "b3r": b3r, "iota": iota,
            "meta": g["meta"][k], "mtt": g["mtt"][k],
        })

    res = run_bass_kernel_spmd(nc, in_maps, core_ids=list(range(NCORES)))
    outf = np.zeros((N, OUT_C), np.float32)
    for k in range(NCORES):
        outf[k * npc:(k + 1) * npc] = res.results[k]["out"][:npc]
    return outf

